# revision 1
# baseline (speedup 1.0000x reference)
"""GQA attention kernel for 8 Trainium2 cores.

Problem: B=2, T=2048, D=2048, 32 q-heads, 8 kv-heads, head_dim=64, causal.

Sharding: core c = (b, jg) with b = c//4, jg = c%4. Each core handles batch b,
kv-heads {2jg, 2jg+1} and q-heads {8jg..8jg+7} (data parallel on B, tensor
parallel on heads; wq/wk/wv column-sharded, wo row-sharded). Each core returns
a partial output projection resT [D, T]; the host sums the 4 partials per
batch and transposes.

Device-side design (per core):
 - qkv projections computed in [t, f] layout (lhsT = x^T k-tiles, rhs = w^T),
   RoPE applied in [t, f] layout where the half-swap is a free-dim AP read,
   then PE-transposed to [f, t] for attention.
 - scores computed TRANSPOSED: scoresT[s, t] = k^T.T @ q^T so softmax's
   reduction axis lands where the AV matmul wants it; no P transposes.
 - softmax skips the row-max (scores are bounded ~|q||k|/8 << 88) so
   P = exp(scores/8) directly out of PSUM via one ACT pass.
 - causal: only lower s-blocks computed; diagonal blocks masked by a
   multiplicative tril pattern built on device.
 - denominator: a ones-column appended to V makes AV emit row 64 = sum_s P.
   reciprocal via ACT Ln/Exp; broadcast across partitions via a DRAM
   round-trip (partition-step-0 read); applied as one tensor_tensor mult.
"""

import os
import sys

sys.path.insert(0, "/opt/trn_rl_repo")

import json

import numpy as np
import ml_dtypes

import concourse.bass as bass
import concourse.mybir as mybir
from concourse.tile import TileContext
from concourse.masks import make_identity
from concourse.bass_utils import run_bass_kernel_spmd

BF16 = mybir.dt.bfloat16
F32 = mybir.dt.float32

T = 2048
D = 2048
HD = 64
NCORES = 8
KT = D // 128          # 16 contraction tiles
NTT = T // 128         # 16 t tiles
NCH = T // 512         # 4 t chunks
NBF = ml_dtypes.bfloat16

# ---------------------------------------------------------------------------
# BIR post-pass: split multi-wait instructions into single-wait
# EventSemaphore carriers (the walrus build here allows one wait per inst).
# ---------------------------------------------------------------------------
_ws_ctr = [0]


def _split_waits_bytes(bir: bytes) -> bytes:
    d = json.loads(bir)
    for f in d.get("functions", []):
        for bb in f.get("blocks", []):
            out = []
            for inst in bb.get("instructions", []):
                si = inst.get("sync_info")
                waits = (si or {}).get("on_wait") or []
                if len(waits) > 1:
                    for w in waits[:-1]:
                        _ws_ctr[0] += 1
                        out.append({
                            "debug": inst.get("debug", 0),
                            "engine": inst["engine"],
                            "ins": [],
                            "name": f"WS-{_ws_ctr[0]}",
                            "opcode": "EventSemaphore",
                            "outs": [],
                            "sync_info": {"on_update": [], "on_wait": [w]},
                        })
                    si["on_wait"] = [waits[-1]]
                out.append(inst)
            bb["instructions"] = out
    return json.dumps(d).encode()


def _install_waitsplit():
    import concourse.bass2jax as b2j

    if getattr(b2j, "_waitsplit_installed", False):
        return
    orig = b2j._decompress_ant_bir
    b2j._decompress_ant_bir = lambda s: _split_waits_bytes(orig(s))
    b2j._waitsplit_installed = True


# ---------------------------------------------------------------------------
# Device program
# ---------------------------------------------------------------------------

def _bcast(ap2d, nh):
    """Insert a step-0 head dim into a [p, w] AP -> [p, nh, w]."""
    return bass.AP(tensor=ap2d.tensor, offset=ap2d.offset,
                   ap=[ap2d.ap[0], [0, nh], ap2d.ap[1]])


def _build(causal: bool):
    nc = bass.Bass()
    xt = nc.dram_tensor("xt", [D, T], BF16, kind="ExternalInput")
    wq = nc.dram_tensor("wq", [D, 512], BF16, kind="ExternalInput")
    wk = nc.dram_tensor("wk", [D, 128], BF16, kind="ExternalInput")
    wv = nc.dram_tensor("wv", [D, 128], BF16, kind="ExternalInput")
    wo = nc.dram_tensor("wo", [512, D], BF16, kind="ExternalInput")
    cexp = nc.dram_tensor("cexp", [T, 64], F32, kind="ExternalInput")
    sexp = nc.dram_tensor("sexp", [T, 64], F32, kind="ExternalInput")
    res = nc.dram_tensor("res", [D, T], F32, kind="ExternalOutput")

    with TileContext(nc) as tc:
        with (
            tc.tile_pool(name="const", bufs=1) as const,
            tc.tile_pool(name="big", bufs=1) as big,
            tc.tile_pool(name="work", bufs=3) as work,
            tc.tile_pool(name="qtfp", bufs=4) as qtfp,
            tc.tile_pool(name="qpool", bufs=5) as qpool,
            tc.tile_pool(name="ptp", bufs=6) as ptp,
            tc.tile_pool(name="ohp", bufs=3) as ohp,
            tc.tile_pool(name="outp", bufs=5) as outp,
            tc.tile_pool(name="scrp", bufs=4, space="DRAM") as scrp,
            tc.tile_pool(name="pmm", bufs=2, space="PSUM") as pmm,
            tc.tile_pool(name="psc", bufs=2, space="PSUM") as psc,
            tc.tile_pool(name="poh", bufs=2, space="PSUM") as poh,
        ):
            # ---------------- constants + weights ----------------
            ident = const.tile([128, 128], BF16)
            make_identity(nc, ident)

            cexp_sb = const.tile([128, NTT, 64], F32)
            sexp_sb = const.tile([128, NTT, 64], F32)
            nc.sync.dma_start(out=cexp_sb, in_=cexp.rearrange("(n p) c -> p n c", p=128))
            nc.sync.dma_start(out=sexp_sb, in_=sexp.rearrange("(n p) c -> p n c", p=128))

            mdiag = None
            if causal:
                mdiag = const.tile([128, 4, 512], BF16)
                nc.vector.memset(mdiag, 1.0)
                for r in range(4):
                    nc.gpsimd.affine_select(
                        out=mdiag[:, r, :], in_=mdiag[:, r, :],
                        pattern=[[1, 512]], base=-128 * r, channel_multiplier=-1,
                        compare_op=mybir.AluOpType.is_ge, fill=0.0)

            xt_sb = big.tile([128, KT, T], BF16)
            for kt in range(KT):
                nc.sync.dma_start(out=xt_sb[:, kt, :], in_=xt[kt * 128:(kt + 1) * 128, :])
            wq_sb = big.tile([128, KT, 512], BF16)
            wk_sb = big.tile([128, KT, 128], BF16)
            wv_sb = big.tile([128, KT, 128], BF16)
            for kt in range(KT):
                nc.sync.dma_start(out=wq_sb[:, kt, :], in_=wq[kt * 128:(kt + 1) * 128, :])
            nc.sync.dma_start(out=wk_sb, in_=wk.rearrange("(n p) c -> p n c", p=128))
            nc.sync.dma_start(out=wv_sb, in_=wv.rearrange("(n p) c -> p n c", p=128))
            wo_sb = big.tile([128, 4, D], BF16)
            for g in range(4):
                nc.sync.dma_start(out=wo_sb[:, g, :], in_=wo[g * 128:(g + 1) * 128, :])

            kT_sb = big.tile([128, NTT, 128], BF16)
            vp_a = big.tile([128, NTT, 65], BF16)
            vp_b = big.tile([128, NTT, 65], BF16)
            nc.vector.memset(vp_a[:, :, 64:65], 1.0)
            nc.vector.memset(vp_b[:, :, 64:65], 1.0)

            def rope(ps, out_bf, ti, nh):
                """RoPE in [t, f] layout. ps: PSUM [128, nh*64] f32 ->
                out_bf: SBUF [128, nh*64] bf16."""
                ps3 = ps.rearrange("p (h k) -> p h k", h=nh)
                o3 = out_bf.rearrange("p (h k) -> p h k", h=nh)
                a = work.tile([128, nh, 64], F32, tag="ropeA", name="ra")
                bt = work.tile([128, nh, 64], F32, tag="ropeB", name="rb")
                cb = _bcast(cexp_sb[:, ti, :], nh)
                nc.vector.tensor_tensor(out=a, in0=ps3, in1=cb, op=mybir.AluOpType.mult)
                sb_lo = _bcast(sexp_sb[:, ti, 0:32], nh)
                sb_hi = _bcast(sexp_sb[:, ti, 32:64], nh)
                nc.vector.tensor_tensor(out=bt[:, :, 0:32], in0=ps3[:, :, 32:64],
                                        in1=sb_lo, op=mybir.AluOpType.mult)
                nc.vector.tensor_tensor(out=bt[:, :, 32:64], in0=ps3[:, :, 0:32],
                                        in1=sb_hi, op=mybir.AluOpType.mult)
                nc.vector.tensor_tensor(out=o3, in0=a, in1=bt, op=mybir.AluOpType.add)

            # ---------------- k/v projection + rope + transpose ----------------
            for st in range(NTT):
                ps_k = pmm.tile([128, 512], F32, tag="mm", name="psk")
                for kt in range(KT):
                    nc.tensor.matmul(ps_k[:, 0:128], xt_sb[:, kt, st * 128:(st + 1) * 128],
                                     wk_sb[:, kt, :], start=(kt == 0), stop=(kt == KT - 1))
                ktf = work.tile([128, 128], BF16, tag="ktf", name="ktf")
                rope(ps_k[:, 0:128], ktf, st, 2)
                pt_k = psc.tile([128, 512], BF16, tag="sc", name="ptk")
                nc.tensor.transpose(pt_k[:, 0:128], ktf, ident)
                nc.vector.tensor_copy(kT_sb[:, st, :], pt_k[:, 0:128])

                ps_v = pmm.tile([128, 512], F32, tag="mm", name="psv")
                for kt in range(KT):
                    nc.tensor.matmul(ps_v[:, 0:128], xt_sb[:, kt, st * 128:(st + 1) * 128],
                                     wv_sb[:, kt, :], start=(kt == 0), stop=(kt == KT - 1))
                nc.vector.tensor_copy(vp_a[:, st, 0:64], ps_v[:, 0:64])
                nc.vector.tensor_copy(vp_b[:, st, 0:64], ps_v[:, 64:128])

            # ---------------- q projection + rope + transpose (all chunks) ----
            qT_all = big.tile([128, NTT, 512], BF16)
            for ti in range(NTT):
                ps_q = pmm.tile([128, 512], F32, tag="mm", name="psq")
                for kt in range(KT):
                    nc.tensor.matmul(ps_q, xt_sb[:, kt, ti * 128:(ti + 1) * 128],
                                     wq_sb[:, kt, :], start=(kt == 0), stop=(kt == KT - 1))
                qt = qtfp.tile([128, 512], BF16, tag="qtf", name="qtf")
                rope(ps_q, qt, ti, 8)
                for g in range(4):
                    pt_q = psc.tile([128, 512], BF16, tag="sc", name="ptq")
                    nc.tensor.transpose(pt_q[:, 0:128],
                                        qt[:, g * 128:(g + 1) * 128], ident)
                    j, tt = ti // 4, ti % 4
                    nc.vector.tensor_copy(
                        qT_all[:, 4 * j + g, tt * 128:(tt + 1) * 128], pt_q[:, 0:128])

            # ---------------- per t-chunk attention + output proj ----------
            for j in range(NCH):
                qTs = [qT_all[:, 4 * j + g, :] for g in range(4)]
                ns = 4 * j + 4 if causal else NTT
                ohn = []
                for g in range(4):
                    po_a = poh.tile([65, 512], F32, tag="oh", name="poa")
                    po_b = poh.tile([65, 512], F32, tag="oh", name="pob")
                    for sp in range(0, ns, 2):
                        for half, po, vp in ((0, po_a, vp_a), (1, po_b, vp_b)):
                            ps2 = psc.tile([128, 2, 512], F32, tag="sc", name="pss")
                            for u in (0, 1):
                                nc.tensor.matmul(
                                    ps2[:, u, :],
                                    kT_sb[half * 64:(half + 1) * 64, sp + u, :],
                                    qTs[g][half * 64:(half + 1) * 64, :],
                                    start=True, stop=True, skip_group_check=True)
                            pt2 = ptp.tile([128, 2, 512], BF16, tag="pt", name="pt")
                            nc.scalar.activation(out=pt2, in_=ps2,
                                                 func=mybir.ActivationFunctionType.Exp,
                                                 scale=0.125)
                            for u in (0, 1):
                                si = sp + u
                                if causal and si >= 4 * j:
                                    nc.vector.tensor_tensor(
                                        out=pt2[:, u, :], in0=pt2[:, u, :],
                                        in1=mdiag[:, si - 4 * j, :],
                                        op=mybir.AluOpType.mult)
                                nc.tensor.matmul(po, vp[:, si, 0:65], pt2[:, u, :],
                                                 start=(si == 0), stop=(si == ns - 1),
                                                 skip_group_check=True)

                    ohn_g = outp.tile([128, 512], BF16, tag="ohn", name="ohn")
                    for half, po in ((0, po_a), (1, po_b)):
                        rt = work.tile([65, 512], F32, tag="rt", name="rt")
                        nc.scalar.activation(out=rt[64:65, :], in_=po[64:65, :],
                                             func=mybir.ActivationFunctionType.Ln)
                        nc.scalar.activation(out=rt[64:65, :], in_=rt[64:65, :],
                                             func=mybir.ActivationFunctionType.Exp,
                                             scale=-1.0)
                        ohf = ohp.tile([65, 512], F32, tag="ohf", name="ohf")
                        nc.vector.tensor_copy(ohf[0:64, :], po[0:64, :])
                        scr = scrp.tile([1, 512], F32, tag="scr", name="scr")
                        nc.gpsimd.dma_start(out=scr, in_=rt[64:65, :])
                        rbc = work.tile([64, 512], F32, tag="rbc", name="rbc")
                        nc.gpsimd.dma_start(
                            out=rbc,
                            in_=bass.AP(tensor=scr.tensor, offset=scr.offset,
                                        ap=[[0, 64], scr.ap[-1]]))
                        if half == 0:
                            nc.vector.tensor_tensor(out=ohn_g[0:64, :], in0=ohf[0:64, :],
                                                    in1=rbc, op=mybir.AluOpType.mult)
                        else:
                            ohnb = work.tile([64, 512], BF16, tag="ohnb", name="ohnb")
                            nc.vector.tensor_tensor(out=ohnb, in0=ohf[0:64, :],
                                                    in1=rbc, op=mybir.AluOpType.mult)
                            nc.gpsimd.dma_start(out=ohn_g[64:128, :], in_=ohnb)
                    ohn.append(ohn_g)

                # output projection for this t-chunk
                for jt in range(NTT):
                    ps_r = pmm.tile([128, 512], F32, tag="mm", name="psr")
                    for g in range(4):
                        nc.tensor.matmul(ps_r, wo_sb[:, g, jt * 128:(jt + 1) * 128],
                                         ohn[g], start=(g == 0), stop=(g == 3),
                                         skip_group_check=True)
                    rs = outp.tile([128, 512], F32, tag="rs", name="rs")
                    nc.vector.tensor_copy(rs, ps_r)
                    nc.sync.dma_start(
                        out=res[jt * 128:(jt + 1) * 128, j * 512:(j + 1) * 512], in_=rs)
    return nc


_NC_CACHE = {}


def _get_nc(causal: bool):
    if causal not in _NC_CACHE:
        _NC_CACHE[causal] = _build(causal)
    return _NC_CACHE[causal]


# ---------------------------------------------------------------------------
# Host wrapper
# ---------------------------------------------------------------------------

def kernel(x, cos, sin, mask, wq, wk, wv, wo):
    x = np.asarray(x, dtype=np.float32)
    cos = np.asarray(cos, dtype=np.float32)
    sin = np.asarray(sin, dtype=np.float32)
    mask = np.asarray(mask)
    wq = np.asarray(wq, dtype=np.float32)
    wk = np.asarray(wk, dtype=np.float32)
    wv = np.asarray(wv, dtype=np.float32)
    wo = np.asarray(wo, dtype=np.float32)

    m2 = mask[0, 0]
    tril = np.tril(np.ones((T, T), dtype=bool))
    if np.array_equal(m2, tril):
        causal = True
    elif m2.all():
        causal = False
    else:
        return _numpy_fallback(x, cos, sin, mask, wq, wk, wv, wo)

    _install_waitsplit()
    nc = _get_nc(causal)

    cexp = np.concatenate([cos, cos], axis=1).astype(np.float32)
    sexp = np.concatenate([-sin, sin], axis=1).astype(np.float32)

    in_maps = []
    for c in range(NCORES):
        b, jg = c // 4, c % 4
        heads = []
        for g in range(4):
            heads.append(8 * jg + g)
            heads.append(8 * jg + 4 + g)
        wq_rows = np.concatenate([wq[h * HD:(h + 1) * HD, :] for h in heads], axis=0)
        wo_cols = np.concatenate([wo[:, h * HD:(h + 1) * HD].T for h in heads], axis=0)
        kv = [2 * jg, 2 * jg + 1]
        wk_rows = np.concatenate([wk[k * HD:(k + 1) * HD, :] for k in kv], axis=0)
        wv_rows = np.concatenate([wv[k * HD:(k + 1) * HD, :] for k in kv], axis=0)
        in_maps.append({
            "xt": np.ascontiguousarray(x[b].T).astype(NBF),
            "wq": np.ascontiguousarray(wq_rows.T).astype(NBF),
            "wk": np.ascontiguousarray(wk_rows.T).astype(NBF),
            "wv": np.ascontiguousarray(wv_rows.T).astype(NBF),
            "wo": np.ascontiguousarray(wo_cols).astype(NBF),
            "cexp": cexp,
            "sexp": sexp,
        })

    trace = os.environ.get("GQA_TRACE") == "1"
    r = run_bass_kernel_spmd(nc, in_maps, core_ids=list(range(NCORES)), trace=trace)
    if trace:
        print("exec_time_ns:", r.exec_time_ns)

    out = np.zeros((2, T, D), dtype=np.float32)
    for c in range(NCORES):
        out[c // 4] += r.results[c]["res"].T
    return out


def _numpy_fallback(x, cos, sin, mask, wq, wk, wv, wo):
    B = x.shape[0]
    NH, NKV = 32, 8
    q = (x @ wq.T).reshape(B, T, NH, HD).transpose(0, 2, 1, 3)
    k = (x @ wk.T).reshape(B, T, NKV, HD).transpose(0, 2, 1, 3)
    v = (x @ wv.T).reshape(B, T, NKV, HD).transpose(0, 2, 1, 3)

    def rope_np(t4):
        c = cos[None, None]
        s = sin[None, None]
        t1, t2 = t4[..., :32], t4[..., 32:]
        return np.concatenate([t1 * c - t2 * s, t2 * c + t1 * s], axis=-1)

    q, k = rope_np(q), rope_np(k)
    k = np.repeat(k, 4, axis=1)
    v = np.repeat(v, 4, axis=1)
    att = np.einsum("bhtd,bhsd->bhts", q, k) / np.sqrt(HD)
    att = np.where(mask, att, -np.inf)
    att = att - att.max(axis=-1, keepdims=True)
    p = np.exp(att)
    p /= p.sum(axis=-1, keepdims=True)
    o = np.einsum("bhts,bhsd->bhtd", p, v)
    o = o.transpose(0, 2, 1, 3).reshape(B, T, -1)
    return (o @ wo.T).astype(np.float32)



# revision 2
# speedup vs baseline: 1.2676x; 1.2676x over previous
"""GQA attention kernel for 8 Trainium2 cores (v2).

Problem: B=2, T=2048, D=2048, 32 q-heads, 8 kv-heads, head_dim=64, causal.

Sharding: core c = (b, jg) with b = c//4, jg = c%4. Each core handles batch b,
kv-heads {2jg, 2jg+1} and q-heads {8jg..8jg+7} (data parallel on B, tensor
parallel on heads; wq/wk/wv column-sharded, wo row-sharded). Each core returns
a partial output projection resT [D, T]; the host sums the 4 partials per
batch and transposes.

v2 design changes vs the 406us baseline:
 - input DMAs spread over the SP/ACT/POOL queues with wkv first so the first
   projection matmul starts at ~14us instead of ~44us.
 - k/q [t,f]->[f,t] transposes moved off the PE onto the DMA XBAR
   (dma_start_transpose), eliminating PE transpose+copy traffic.
 - AV matmul transposed: out[t(128), f(65)] accumulates with P-tiles as the
   stationary operand and [V|1] as the 65-row moving operand (65 rows vs 512
   rows per s-tile on the PE). The 65th column of the accumulator is the
   softmax denominator, so no separate reduction and no DRAM round-trip
   broadcast: reciprocal via ACT Ln/Exp on a [128,4,1] column, applied as a
   free-dim step-0 broadcast multiply.
 - PSUM accumulators for the 4 t-blocks share one bank; groups cannot use
   start=True (2KB zero-region granularity), so the bank is DVE-memset to 0
   and all AV matmuls accumulate with start=False.
 - causal diagonal computed at 128-column granularity (only the lower
   trapezoid), saving ~37% of diagonal scores/exp/AV work; only the true
   diagonal 128x128 subtiles get the multiplicative tril mask (on gpsimd).
 - output-projection results staged through SBUF (DMA cannot read PSUM) and
   streamed out per 128-row tile on the idle SP/POOL queues.
"""

import os
import sys

sys.path.insert(0, "/opt/trn_rl_repo")

import json

import numpy as np
import ml_dtypes

import concourse.bass as bass
import concourse.mybir as mybir
from concourse.tile import TileContext
from concourse.masks import make_identity
from concourse.bass_utils import run_bass_kernel_spmd

BF16 = mybir.dt.bfloat16
F32 = mybir.dt.float32

T = 2048
D = 2048
HD = 64
NCORES = 8
KT = D // 128           # 16 contraction tiles
NTT = T // 128          # 16 time tiles
NCH = T // 512          # 4 chunks
NBF = ml_dtypes.bfloat16

# ---------------------------------------------------------------------------
# BIR post-pass: split multi-wait instructions into single-wait
# EventSemaphore carriers (the walrus build here allows one wait per inst).
# ---------------------------------------------------------------------------
_ws_ctr = [0]


def _split_waits_bytes(bir: bytes) -> bytes:
    d = json.loads(bir)
    for f in d.get("functions", []):
        for bb in f.get("blocks", []):
            out = []
            for inst in bb.get("instructions", []):
                si = inst.get("sync_info")
                waits = (si or {}).get("on_wait") or []
                if len(waits) > 1:
                    for w in waits[:-1]:
                        _ws_ctr[0] += 1
                        out.append({
                            "debug": inst.get("debug", 0),
                            "engine": inst["engine"],
                            "ins": [],
                            "name": f"WS-{_ws_ctr[0]}",
                            "opcode": "EventSemaphore",
                            "outs": [],
                            "sync_info": {"on_update": [], "on_wait": [w]},
                        })
                    si["on_wait"] = [waits[-1]]
                out.append(inst)
            bb["instructions"] = out
    return json.dumps(d).encode()


def _install_waitsplit():
    import concourse.bass2jax as b2j

    if getattr(b2j, "_waitsplit_installed", False):
        return
    orig = b2j._decompress_ant_bir
    b2j._decompress_ant_bir = lambda s: _split_waits_bytes(orig(s))
    b2j._waitsplit_installed = True


# ---------------------------------------------------------------------------
# Device program
# ---------------------------------------------------------------------------

def _bcast(ap2d, nh):
    """Insert a step-0 head dim into a [p, w] AP -> [p, nh, w]."""
    return bass.AP(tensor=ap2d.tensor, offset=ap2d.offset,
                   ap=[ap2d.ap[0], [0, nh], ap2d.ap[1]])


def _bcast_last(ap3d, w):
    """Append a step-0 last dim to a [p, n, 1] AP -> [p, n, w]."""
    return bass.AP(tensor=ap3d.tensor, offset=ap3d.offset,
                   ap=[ap3d.ap[0], ap3d.ap[1], [0, w]])


def _build(causal: bool):
    nc = bass.Bass()
    xt = nc.dram_tensor("xt", [D, T], BF16, kind="ExternalInput")
    wq = nc.dram_tensor("wq", [D, 512], BF16, kind="ExternalInput")
    wkv = nc.dram_tensor("wkv", [D, 256], BF16, kind="ExternalInput")
    wo = nc.dram_tensor("wo", [512, D], BF16, kind="ExternalInput")
    cexp = nc.dram_tensor("cexp", [T, 64], F32, kind="ExternalInput")
    sexp = nc.dram_tensor("sexp", [T, 64], F32, kind="ExternalInput")
    res = nc.dram_tensor("res", [D, T], F32, kind="ExternalOutput")

    with TileContext(nc) as tc:
        with (
            tc.tile_pool(name="const", bufs=1) as const,
            tc.tile_pool(name="big", bufs=1) as big,
            tc.tile_pool(name="ropew", bufs=3) as ropew,
            tc.tile_pool(name="qtfp", bufs=3) as qtfp,
            tc.tile_pool(name="ptp", bufs=4) as ptp,
            tc.tile_pool(name="ohp", bufs=3) as ohp,
            tc.tile_pool(name="rp", bufs=3) as rp,
            tc.tile_pool(name="ohtp", bufs=6) as ohtp,
            tc.tile_pool(name="rsp", bufs=3) as rsp,
            tc.tile_pool(name="pmm", bufs=2, space="PSUM") as pmm,
            tc.tile_pool(name="psc", bufs=2, space="PSUM") as psc,
            tc.tile_pool(name="pav", bufs=2, space="PSUM") as pavp,
        ):
            # ---------------- constants ----------------
            ident = const.tile([128, 128], BF16)
            make_identity(nc, ident)

            mtri = None
            if causal:
                # keep col >= row (upper triangle incl diagonal) of a
                # [s_local, t_local] 128x128 tile
                mtri = const.tile([128, 128], BF16)
                nc.vector.memset(mtri, 1.0)
                nc.gpsimd.affine_select(
                    out=mtri, in_=mtri, pattern=[[1, 128]], base=0,
                    channel_multiplier=-1, compare_op=mybir.AluOpType.is_ge,
                    fill=0.0)

            # ---------------- weights + x loads, spread over 3 queues ------
            wkv_sb = big.tile([128, KT, 256], BF16)
            nc.sync.dma_start(out=wkv_sb, in_=wkv.rearrange("(n p) c -> p n c", p=128))

            cexp_sb = const.tile([128, NTT, 64], F32)
            sexp_sb = const.tile([128, NTT, 64], F32)
            nc.scalar.dma_start(out=cexp_sb, in_=cexp.rearrange("(n p) c -> p n c", p=128))
            nc.scalar.dma_start(out=sexp_sb, in_=sexp.rearrange("(n p) c -> p n c", p=128))

            wq_sb = big.tile([128, KT, 512], BF16)
            nc.gpsimd.dma_start(out=wq_sb, in_=wq.rearrange("(n p) c -> p n c", p=128))

            xt_sb = big.tile([128, KT, T], BF16)
            xq = [nc.sync, nc.scalar, nc.gpsimd]
            for kt in range(KT):
                xq[kt % 3].dma_start(out=xt_sb[:, kt, :],
                                     in_=xt[kt * 128:(kt + 1) * 128, :])

            wo_sb = big.tile([128, 4, D], BF16)
            nc.scalar.dma_start(out=wo_sb, in_=wo.rearrange("(g p) d -> p g d", p=128))

            kT_sb = big.tile([128, NTT, 128], BF16)
            qT_all = big.tile([128, NTT, 512], BF16)
            vp_a = big.tile([128, NTT, 65], BF16)
            vp_b = big.tile([128, NTT, 65], BF16)
            nc.vector.memset(vp_a[:, :, 64:65], 1.0)
            nc.vector.memset(vp_b[:, :, 64:65], 1.0)

            def rope(ps3, out_bf, ti, nh):
                """RoPE in [t, f] layout. ps3: PSUM [128, nh, 64] f32 view ->
                out_bf: SBUF [128, nh*64] bf16."""
                o3 = out_bf.rearrange("p (h k) -> p h k", h=nh)
                a = ropew.tile([128, nh, 64], F32, tag="ropeA", name="ra")
                bt = ropew.tile([128, nh, 64], F32, tag="ropeB", name="rb")
                cb = _bcast(cexp_sb[:, ti, :], nh)
                nc.vector.tensor_tensor(out=a, in0=ps3, in1=cb, op=mybir.AluOpType.mult)
                sb_lo = _bcast(sexp_sb[:, ti, 0:32], nh)
                sb_hi = _bcast(sexp_sb[:, ti, 32:64], nh)
                nc.vector.tensor_tensor(out=bt[:, :, 0:32], in0=ps3[:, :, 32:64],
                                        in1=sb_lo, op=mybir.AluOpType.mult)
                nc.vector.tensor_tensor(out=bt[:, :, 32:64], in0=ps3[:, :, 0:32],
                                        in1=sb_hi, op=mybir.AluOpType.mult)
                # SBUF-only add on gpsimd to offload DVE
                nc.gpsimd.tensor_tensor(out=o3, in0=a, in1=bt, op=mybir.AluOpType.add)

            # ---------------- phase A: kv + q projections, interleaved -----
            for c in range(NCH):
                for st in range(4 * c, 4 * c + 4):
                    ps_kv = pmm.tile([128, 256], F32, tag="mm", name="pskv")
                    for kt in range(KT):
                        nc.tensor.matmul(ps_kv, xt_sb[:, kt, st * 128:(st + 1) * 128],
                                         wkv_sb[:, kt, :], start=(kt == 0),
                                         stop=(kt == KT - 1))
                    ktf = qtfp.tile([128, 128], BF16, tag="qtf", name="ktf")
                    rope(ps_kv[:, 0:128].rearrange("p (h k) -> p h k", h=2),
                         ktf, st, 2)
                    nc.sync.dma_start_transpose(kT_sb[:, st, :], ktf)
                    nc.vector.tensor_copy(vp_a[:, st, 0:64], ps_kv[:, 128:192])
                    nc.vector.tensor_copy(vp_b[:, st, 0:64], ps_kv[:, 192:256])
                for ti in range(4 * c, 4 * c + 4):
                    ps_q = pmm.tile([128, 512], F32, tag="mm", name="psq")
                    for kt in range(KT):
                        nc.tensor.matmul(ps_q, xt_sb[:, kt, ti * 128:(ti + 1) * 128],
                                         wq_sb[:, kt, :], start=(kt == 0),
                                         stop=(kt == KT - 1))
                    qtf = qtfp.tile([128, 512], BF16, tag="qtf", name="qtf")
                    rope(ps_q.rearrange("p (h k) -> p h k", h=8), qtf, ti, 8)
                    tt = ti % 4
                    for g in range(4):
                        nc.sync.dma_start_transpose(
                            qT_all[:, 4 * c + g, tt * 128:(tt + 1) * 128],
                            qtf[:, g * 128:(g + 1) * 128])

            # ---------------- phase B: attention + output projection -------
            for j in range(NCH):
                ohT = []
                for g in range(4):
                    qT = qT_all[:, 4 * j + g, :]
                    poht = pmm.tile([128, 4, 128], BF16, tag="mm", name="poht")
                    for half, vp in ((0, vp_a), (1, vp_b)):
                        hb = 64 * half
                        pav = pavp.tile([128, 4, 128], F32, tag="av", name="pav")
                        nc.vector.memset(pav[:, :, 0:65], 0.0)

                        def av(ptile, si, tb0):
                            for tb in range(tb0, 4):
                                nc.tensor.matmul(
                                    pav[:, tb, 0:65],
                                    ptile[:, (tb - tb0) * 128:(tb - tb0 + 1) * 128],
                                    vp[:, si, :], start=False, stop=False,
                                    skip_group_check=True)

                        ns_off = 4 * j if causal else NTT
                        for sp in range(0, ns_off, 2):
                            ps2 = psc.tile([128, 2, 512], F32, tag="sc", name="pss")
                            for u in (0, 1):
                                nc.tensor.matmul(
                                    ps2[:, u, :], kT_sb[hb:hb + 64, sp + u, :],
                                    qT[hb:hb + 64, :],
                                    start=True, stop=True, skip_group_check=True)
                            pt2 = ptp.tile([128, 2, 512], BF16, tag="pt", name="pt")
                            nc.scalar.activation(out=pt2, in_=ps2,
                                                 func=mybir.ActivationFunctionType.Exp,
                                                 scale=0.125)
                            for u in (0, 1):
                                av(pt2[:, u, :], sp + u, 0)

                        if causal:
                            s0 = 4 * j
                            psd1 = psc.tile([128, 2, 512], F32, tag="sc", name="psd1")
                            psd2 = psc.tile([128, 2, 512], F32, tag="sc", name="psd2")
                            nc.tensor.matmul(psd1[:, 0, :],
                                             kT_sb[hb:hb + 64, s0, :],
                                             qT[hb:hb + 64, :],
                                             start=True, stop=True,
                                             skip_group_check=True)
                            nc.tensor.matmul(psd1[:, 1, 0:384],
                                             kT_sb[hb:hb + 64, s0 + 1, :],
                                             qT[hb:hb + 64, 128:512],
                                             start=True, stop=True,
                                             skip_group_check=True)
                            nc.tensor.matmul(psd1[:, 1, 384:512],
                                             kT_sb[hb:hb + 64, s0 + 3, :],
                                             qT[hb:hb + 64, 384:512],
                                             start=True, stop=True,
                                             skip_group_check=True)
                            nc.tensor.matmul(psd2[:, 0, 0:256],
                                             kT_sb[hb:hb + 64, s0 + 2, :],
                                             qT[hb:hb + 64, 256:512],
                                             start=True, stop=True,
                                             skip_group_check=True)
                            pd1 = ptp.tile([128, 2, 512], BF16, tag="pt", name="pd1")
                            nc.scalar.activation(out=pd1, in_=psd1,
                                                 func=mybir.ActivationFunctionType.Exp,
                                                 scale=0.125)
                            pd2 = ptp.tile([128, 2, 512], BF16, tag="pt", name="pd2")
                            nc.scalar.activation(out=pd2[:, 0, 0:256],
                                                 in_=psd2[:, 0, 0:256],
                                                 func=mybir.ActivationFunctionType.Exp,
                                                 scale=0.125)
                            for msk in (pd1[:, 0, 0:128], pd1[:, 1, 0:128],
                                        pd1[:, 1, 384:512], pd2[:, 0, 0:128]):
                                nc.gpsimd.tensor_tensor(out=msk, in0=msk, in1=mtri,
                                                        op=mybir.AluOpType.mult)
                            av(pd1[:, 0, :], s0, 0)
                            av(pd1[:, 1, 0:384], s0 + 1, 1)
                            av(pd2[:, 0, 0:256], s0 + 2, 2)
                            av(pd1[:, 1, 384:512], s0 + 3, 3)

                        # softmax denominator -> reciprocal -> normalize
                        r4 = rp.tile([128, 4, 1], F32, tag="r4", name="r4")
                        nc.scalar.activation(out=r4, in_=pav[:, :, 64:65],
                                             func=mybir.ActivationFunctionType.Ln)
                        nc.scalar.activation(out=r4, in_=r4,
                                             func=mybir.ActivationFunctionType.Exp,
                                             scale=-1.0)
                        ohn = ohp.tile([128, 4, 64], BF16, tag="ohn", name="ohn")
                        nc.vector.tensor_tensor(out=ohn, in0=pav[:, :, 0:64],
                                                in1=_bcast_last(r4, 64),
                                                op=mybir.AluOpType.mult)
                        for tb in range(4):
                            nc.tensor.transpose(poht[hb:hb + 64, tb, :],
                                                ohn[:, tb, :], ident)
                    ohT_g = ohtp.tile([128, 512], BF16, tag="oht", name="oht")
                    nc.vector.tensor_copy(ohT_g.rearrange("p (a b) -> p a b", a=4),
                                          poht)
                    ohT.append(ohT_g)

                # output projection for this t-chunk
                for jt in range(NTT):
                    ps_r = pmm.tile([128, 512], F32, tag="mm", name="psr")
                    for g in range(4):
                        nc.tensor.matmul(ps_r, wo_sb[:, g, jt * 128:(jt + 1) * 128],
                                         ohT[g], start=(g == 0), stop=(g == 3),
                                         skip_group_check=True)
                    rs = rsp.tile([128, 512], F32, tag="rs", name="rs")
                    nc.vector.tensor_copy(rs, ps_r)
                    eng = nc.sync if jt % 2 == 0 else nc.gpsimd
                    eng.dma_start(
                        out=res[jt * 128:(jt + 1) * 128, j * 512:(j + 1) * 512],
                        in_=rs)
    return nc


_NC_CACHE = {}


def _get_nc(causal: bool):
    if causal not in _NC_CACHE:
        _NC_CACHE[causal] = _build(causal)
    return _NC_CACHE[causal]


# ---------------------------------------------------------------------------
# Host wrapper
# ---------------------------------------------------------------------------

def kernel(x, cos, sin, mask, wq, wk, wv, wo):
    x = np.asarray(x, dtype=np.float32)
    cos = np.asarray(cos, dtype=np.float32)
    sin = np.asarray(sin, dtype=np.float32)
    mask = np.asarray(mask)
    wq = np.asarray(wq, dtype=np.float32)
    wk = np.asarray(wk, dtype=np.float32)
    wv = np.asarray(wv, dtype=np.float32)
    wo = np.asarray(wo, dtype=np.float32)

    m2 = mask[0, 0]
    tril = np.tril(np.ones((T, T), dtype=bool))
    if np.array_equal(m2, tril):
        causal = True
    elif m2.all():
        causal = False
    else:
        return _numpy_fallback(x, cos, sin, mask, wq, wk, wv, wo)

    _install_waitsplit()
    nc = _get_nc(causal)

    cexp = np.concatenate([cos, cos], axis=1).astype(np.float32)
    sexp = np.concatenate([-sin, sin], axis=1).astype(np.float32)

    in_maps = []
    for c in range(NCORES):
        b, jg = c // 4, c % 4
        heads = []
        for g in range(4):
            heads.append(8 * jg + g)
            heads.append(8 * jg + 4 + g)
        wq_rows = np.concatenate([wq[h * HD:(h + 1) * HD, :] for h in heads], axis=0)
        wo_cols = np.concatenate([wo[:, h * HD:(h + 1) * HD].T for h in heads], axis=0)
        kv = [2 * jg, 2 * jg + 1]
        wk_rows = np.concatenate([wk[k * HD:(k + 1) * HD, :] for k in kv], axis=0)
        wv_rows = np.concatenate([wv[k * HD:(k + 1) * HD, :] for k in kv], axis=0)
        wkv_cols = np.concatenate([wk_rows.T, wv_rows.T], axis=1)  # [D, 256]
        in_maps.append({
            "xt": np.ascontiguousarray(x[b].T).astype(NBF),
            "wq": np.ascontiguousarray(wq_rows.T).astype(NBF),
            "wkv": np.ascontiguousarray(wkv_cols).astype(NBF),
            "wo": np.ascontiguousarray(wo_cols).astype(NBF),
            "cexp": cexp,
            "sexp": sexp,
        })

    r = run_bass_kernel_spmd(nc, in_maps, core_ids=list(range(NCORES)))

    out = np.zeros((2, T, D), dtype=np.float32)
    for c in range(NCORES):
        out[c // 4] += r.results[c]["res"].T
    return out


def _numpy_fallback(x, cos, sin, mask, wq, wk, wv, wo):
    B = x.shape[0]
    NH, NKV = 32, 8
    q = (x @ wq.T).reshape(B, T, NH, HD).transpose(0, 2, 1, 3)
    k = (x @ wk.T).reshape(B, T, NKV, HD).transpose(0, 2, 1, 3)
    v = (x @ wv.T).reshape(B, T, NKV, HD).transpose(0, 2, 1, 3)

    def rope_np(t4):
        c = cos[None, None]
        s = sin[None, None]
        t1, t2 = t4[..., :32], t4[..., 32:]
        return np.concatenate([t1 * c - t2 * s, t2 * c + t1 * s], axis=-1)

    q, k = rope_np(q), rope_np(k)
    k = np.repeat(k, 4, axis=1)
    v = np.repeat(v, 4, axis=1)
    att = np.einsum("bhtd,bhsd->bhts", q, k) / np.sqrt(HD)
    att = np.where(mask, att, -np.inf)
    att = att - att.max(axis=-1, keepdims=True)
    p = np.exp(att)
    p /= p.sum(axis=-1, keepdims=True)
    o = np.einsum("bhts,bhsd->bhtd", p, v)
    o = o.transpose(0, 2, 1, 3).reshape(B, T, -1)
    return (o @ wo.T).astype(np.float32)


# revision 16
# speedup vs baseline: 1.3013x; 1.0266x over previous
"""GQA attention kernel for 8 Trainium2 cores (v2).

Problem: B=2, T=2048, D=2048, 32 q-heads, 8 kv-heads, head_dim=64, causal.

Sharding: core c = (b, jg) with b = c//4, jg = c%4. Each core handles batch b,
kv-heads {2jg, 2jg+1} and q-heads {8jg..8jg+7} (data parallel on B, tensor
parallel on heads; wq/wk/wv column-sharded, wo row-sharded). Each core returns
a partial output projection resT [D, T]; the host sums the 4 partials per
batch and transposes.

v2 design changes vs the 406us baseline:
 - input DMAs spread over the SP/ACT/POOL queues with wkv first so the first
   projection matmul starts at ~14us instead of ~44us.
 - k/q [t,f]->[f,t] transposes moved off the PE onto the DMA XBAR
   (dma_start_transpose), eliminating PE transpose+copy traffic.
 - AV matmul transposed: out[t(128), f(65)] accumulates with P-tiles as the
   stationary operand and [V|1] as the 65-row moving operand (65 rows vs 512
   rows per s-tile on the PE). The 65th column of the accumulator is the
   softmax denominator, so no separate reduction and no DRAM round-trip
   broadcast: reciprocal via ACT Ln/Exp on a [128,4,1] column, applied as a
   free-dim step-0 broadcast multiply.
 - PSUM accumulators for the 4 t-blocks share one bank; groups cannot use
   start=True (2KB zero-region granularity), so the bank is DVE-memset to 0
   and all AV matmuls accumulate with start=False.
 - causal diagonal computed at 128-column granularity (only the lower
   trapezoid), saving ~37% of diagonal scores/exp/AV work; only the true
   diagonal 128x128 subtiles get the multiplicative tril mask (on gpsimd).
 - output-projection results staged through SBUF (DMA cannot read PSUM) and
   streamed out per 128-row tile on the idle SP/POOL queues.
"""

import os
import sys

sys.path.insert(0, "/opt/trn_rl_repo")

import json

import numpy as np
import ml_dtypes

import concourse.bass as bass
import concourse.mybir as mybir
from concourse.tile import TileContext
from concourse.masks import make_identity
from concourse.bass_utils import run_bass_kernel_spmd

BF16 = mybir.dt.bfloat16
FP8 = mybir.dt.float8e4
F32 = mybir.dt.float32

T = 2048
D = 2048
HD = 64
NCORES = 8
KT = D // 128           # 16 contraction tiles
NTT = T // 128          # 16 time tiles
NCH = T // 512          # 4 chunks
NBF = ml_dtypes.bfloat16

# ---------------------------------------------------------------------------
# BIR post-pass: split multi-wait instructions into single-wait
# EventSemaphore carriers (the walrus build here allows one wait per inst).
# ---------------------------------------------------------------------------
_ws_ctr = [0]


def _split_waits_bytes(bir: bytes) -> bytes:
    d = json.loads(bir)
    for f in d.get("functions", []):
        for bb in f.get("blocks", []):
            out = []
            for inst in bb.get("instructions", []):
                si = inst.get("sync_info")
                waits = (si or {}).get("on_wait") or []
                if len(waits) > 1:
                    for w in waits[:-1]:
                        _ws_ctr[0] += 1
                        out.append({
                            "debug": inst.get("debug", 0),
                            "engine": inst["engine"],
                            "ins": [],
                            "name": f"WS-{_ws_ctr[0]}",
                            "opcode": "EventSemaphore",
                            "outs": [],
                            "sync_info": {"on_update": [], "on_wait": [w]},
                        })
                    si["on_wait"] = [waits[-1]]
                out.append(inst)
            bb["instructions"] = out
    return json.dumps(d).encode()


def _install_waitsplit():
    import concourse.bass2jax as b2j

    if getattr(b2j, "_waitsplit_installed", False):
        return
    orig = b2j._decompress_ant_bir
    b2j._decompress_ant_bir = lambda s: _split_waits_bytes(orig(s))
    b2j._waitsplit_installed = True


# ---------------------------------------------------------------------------
# Device program
# ---------------------------------------------------------------------------

def _bcast(ap2d, nh):
    """Insert a step-0 head dim into a [p, w] AP -> [p, nh, w]."""
    return bass.AP(tensor=ap2d.tensor, offset=ap2d.offset,
                   ap=[ap2d.ap[0], [0, nh], ap2d.ap[1]])


def _bcast_last(ap3d, w):
    """Append a step-0 last dim to a [p, n, 1] AP -> [p, n, w]."""
    return bass.AP(tensor=ap3d.tensor, offset=ap3d.offset,
                   ap=[ap3d.ap[0], ap3d.ap[1], [0, w]])


def _build(causal: bool):
    nc = bass.Bass()
    xt = nc.dram_tensor("xt", [D, T], BF16, kind="ExternalInput")
    wq = nc.dram_tensor("wq", [D, 512], BF16, kind="ExternalInput")
    wkv = nc.dram_tensor("wkv", [D, 256], BF16, kind="ExternalInput")
    wo = nc.dram_tensor("wo", [512, D], BF16, kind="ExternalInput")
    cexp = nc.dram_tensor("cexp", [T, 64], F32, kind="ExternalInput")
    sexp = nc.dram_tensor("sexp", [T, 64], F32, kind="ExternalInput")
    res = nc.dram_tensor("res", [D, T], F32, kind="ExternalOutput")

    with TileContext(nc) as tc:
        with (
            tc.tile_pool(name="const", bufs=1) as const,
            tc.tile_pool(name="big", bufs=1) as big,
            tc.tile_pool(name="ropew", bufs=3) as ropew,
            tc.tile_pool(name="qtfp", bufs=3) as qtfp,
            tc.tile_pool(name="ptp", bufs=4) as ptp,
            tc.tile_pool(name="ohp", bufs=3) as ohp,
            tc.tile_pool(name="rp", bufs=3) as rp,
            tc.tile_pool(name="ohtp", bufs=6) as ohtp,
            tc.tile_pool(name="rsp", bufs=3) as rsp,
            tc.tile_pool(name="pmm", bufs=2, space="PSUM") as pmm,
            tc.tile_pool(name="psc", bufs=2, space="PSUM") as psc,
            tc.tile_pool(name="pav", bufs=2, space="PSUM") as pavp,
        ):
            # ---------------- constants ----------------
            ident = const.tile([128, 128], BF16)
            make_identity(nc, ident)

            mtri = None
            if causal:
                # keep col >= row (upper triangle incl diagonal) of a
                # [s_local, t_local] 128x128 tile
                mtri = const.tile([128, 128], BF16)
                nc.vector.memset(mtri, 1.0)
                nc.gpsimd.affine_select(
                    out=mtri, in_=mtri, pattern=[[1, 128]], base=0,
                    channel_multiplier=-1, compare_op=mybir.AluOpType.is_ge,
                    fill=0.0)

            # ---------------- weights + x loads, spread over 3 queues ------
            wkv_sb = big.tile([128, KT, 256], BF16)
            nc.sync.dma_start(out=wkv_sb, in_=wkv.rearrange("(n p) c -> p n c", p=128))

            cexp_sb = const.tile([128, NTT, 64], F32)
            sexp_sb = const.tile([128, NTT, 64], F32)
            nc.scalar.dma_start(out=cexp_sb, in_=cexp.rearrange("(n p) c -> p n c", p=128))
            nc.scalar.dma_start(out=sexp_sb, in_=sexp.rearrange("(n p) c -> p n c", p=128))

            wq_sb = big.tile([128, KT, 512], BF16)
            nc.gpsimd.dma_start(out=wq_sb, in_=wq.rearrange("(n p) c -> p n c", p=128))

            xt_sb = big.tile([128, KT, T], BF16)
            xq = [nc.sync, nc.scalar, nc.gpsimd]
            for kt in range(KT):
                xq[kt % 3].dma_start(out=xt_sb[:, kt, :],
                                     in_=xt[kt * 128:(kt + 1) * 128, :])

            wo_sb = big.tile([128, 4, D], BF16)
            nc.scalar.dma_start(out=wo_sb, in_=wo.rearrange("(g p) d -> p g d", p=128))

            kT_sb = big.tile([128, NTT, 128], BF16)
            qT_all = big.tile([128, NTT, 512], BF16)
            vp_a = big.tile([128, NTT, 65], BF16)
            vp_b = big.tile([128, NTT, 65], BF16)
            nc.vector.memset(vp_a[:, :, 64:65], 1.0)
            nc.vector.memset(vp_b[:, :, 64:65], 1.0)

            def rope(ps3, out_bf, ti, nh):
                """RoPE in [t, f] layout. ps3: PSUM [128, nh, 64] f32 view ->
                out_bf: SBUF [128, nh*64] bf16."""
                o3 = out_bf.rearrange("p (h k) -> p h k", h=nh)
                a = ropew.tile([128, nh, 64], F32, tag="ropeA", name="ra")
                bt = ropew.tile([128, nh, 64], F32, tag="ropeB", name="rb")
                cb = _bcast(cexp_sb[:, ti, :], nh)
                nc.vector.tensor_tensor(out=a, in0=ps3, in1=cb, op=mybir.AluOpType.mult)
                sb_lo = _bcast(sexp_sb[:, ti, 0:32], nh)
                sb_hi = _bcast(sexp_sb[:, ti, 32:64], nh)
                nc.vector.tensor_tensor(out=bt[:, :, 0:32], in0=ps3[:, :, 32:64],
                                        in1=sb_lo, op=mybir.AluOpType.mult)
                nc.vector.tensor_tensor(out=bt[:, :, 32:64], in0=ps3[:, :, 0:32],
                                        in1=sb_hi, op=mybir.AluOpType.mult)
                # SBUF-only add on gpsimd to offload DVE
                nc.gpsimd.tensor_tensor(out=o3, in0=a, in1=bt, op=mybir.AluOpType.add)

            # ------- projections for one quarter of the time axis ----------
            def proj_quarter(c):
                for st in range(4 * c, 4 * c + 4):
                    ps_kv = pmm.tile([128, 256], F32, tag="mm", name="pskv")
                    for kt in range(KT):
                        nc.tensor.matmul(ps_kv, xt_sb[:, kt, st * 128:(st + 1) * 128],
                                         wkv_sb[:, kt, :], start=(kt == 0),
                                         stop=(kt == KT - 1))
                    ktf = qtfp.tile([128, 128], BF16, tag="qtf", name="ktf")
                    rope(ps_kv[:, 0:128].rearrange("p (h k) -> p h k", h=2),
                         ktf, st, 2)
                    nc.sync.dma_start_transpose(kT_sb[:, st, :], ktf)
                    nc.vector.tensor_copy(vp_a[:, st, 0:64], ps_kv[:, 128:192])
                    nc.vector.tensor_copy(vp_b[:, st, 0:64], ps_kv[:, 192:256])
                for ti in range(4 * c, 4 * c + 4):
                    ps_q = pmm.tile([128, 512], F32, tag="mm", name="psq")
                    for kt in range(KT):
                        nc.tensor.matmul(ps_q, xt_sb[:, kt, ti * 128:(ti + 1) * 128],
                                         wq_sb[:, kt, :], start=(kt == 0),
                                         stop=(kt == KT - 1))
                    qtf = qtfp.tile([128, 512], BF16, tag="qtf", name="qtf")
                    rope(ps_q.rearrange("p (h k) -> p h k", h=8), qtf, ti, 8)
                    tt = ti % 4
                    for g in range(4):
                        nc.sync.dma_start_transpose(
                            qT_all[:, 4 * c + g, tt * 128:(tt + 1) * 128],
                            qtf[:, g * 128:(g + 1) * 128])

            # ------- attention + output projection for one 512-wide chunk --
            def attention_chunk(j):
                ohT_all = ohtp.tile([128, 4, 512], BF16, tag="oht", name="oht")
                for g in range(4):
                    qT = qT_all[:, 4 * j + g, :]
                    poht = pmm.tile([128, 4, 128], BF16, tag="mm", name="poht")
                    for half, vp in ((0, vp_a), (1, vp_b)):
                        hb = 64 * half
                        pav = pavp.tile([128, 4, 128], F32, tag="av", name="pav")
                        nc.vector.memset(pav[:, :, 0:65], 0.0)

                        def av(ptile, si, tb0):
                            for tb in range(tb0, 4):
                                nc.tensor.matmul(
                                    pav[:, tb, 0:65],
                                    ptile[:, (tb - tb0) * 128:(tb - tb0 + 1) * 128],
                                    vp[:, si, :], start=False, stop=False,
                                    skip_group_check=True)

                        ns_off = 4 * j if causal else NTT
                        for sp in range(0, ns_off, 2):
                            ps2 = psc.tile([128, 2, 512], F32, tag="sc", name="pss")
                            for u in (0, 1):
                                nc.tensor.matmul(
                                    ps2[:, u, :], kT_sb[hb:hb + 64, sp + u, :],
                                    qT[hb:hb + 64, :],
                                    start=True, stop=True, skip_group_check=True)
                            pt2 = ptp.tile([128, 2, 512], BF16, tag="pt", name="pt")
                            nc.scalar.activation(out=pt2, in_=ps2,
                                                 func=mybir.ActivationFunctionType.Exp,
                                                 scale=0.125)
                            for u in (0, 1):
                                av(pt2[:, u, :], sp + u, 0)

                        if causal:
                            s0 = 4 * j
                            psd1 = psc.tile([128, 2, 512], F32, tag="sc", name="psd1")
                            psd2 = psc.tile([128, 2, 512], F32, tag="sc", name="psd2")
                            nc.tensor.matmul(psd1[:, 0, :],
                                             kT_sb[hb:hb + 64, s0, :],
                                             qT[hb:hb + 64, :],
                                             start=True, stop=True,
                                             skip_group_check=True)
                            nc.tensor.matmul(psd1[:, 1, 0:384],
                                             kT_sb[hb:hb + 64, s0 + 1, :],
                                             qT[hb:hb + 64, 128:512],
                                             start=True, stop=True,
                                             skip_group_check=True)
                            nc.tensor.matmul(psd1[:, 1, 384:512],
                                             kT_sb[hb:hb + 64, s0 + 3, :],
                                             qT[hb:hb + 64, 384:512],
                                             start=True, stop=True,
                                             skip_group_check=True)
                            nc.tensor.matmul(psd2[:, 0, 0:256],
                                             kT_sb[hb:hb + 64, s0 + 2, :],
                                             qT[hb:hb + 64, 256:512],
                                             start=True, stop=True,
                                             skip_group_check=True)
                            pd1 = ptp.tile([128, 2, 512], BF16, tag="pt", name="pd1")
                            nc.scalar.activation(out=pd1, in_=psd1,
                                                 func=mybir.ActivationFunctionType.Exp,
                                                 scale=0.125)
                            pd2 = ptp.tile([128, 2, 512], BF16, tag="pt", name="pd2")
                            nc.scalar.activation(out=pd2[:, 0, 0:256],
                                                 in_=psd2[:, 0, 0:256],
                                                 func=mybir.ActivationFunctionType.Exp,
                                                 scale=0.125)
                            for msk in (pd1[:, 0, 0:128], pd1[:, 1, 0:128],
                                        pd1[:, 1, 384:512], pd2[:, 0, 0:128]):
                                nc.gpsimd.tensor_tensor(out=msk, in0=msk, in1=mtri,
                                                        op=mybir.AluOpType.mult)
                            av(pd1[:, 0, :], s0, 0)
                            av(pd1[:, 1, 0:384], s0 + 1, 1)
                            av(pd2[:, 0, 0:256], s0 + 2, 2)
                            av(pd1[:, 1, 384:512], s0 + 3, 3)

                        # softmax denominator -> reciprocal -> normalize
                        r4 = rp.tile([128, 4, 1], F32, tag="r4", name="r4")
                        nc.vector.reciprocal(out=r4, in_=pav[:, :, 64:65])
                        ohn = ohp.tile([128, 4, 64], BF16, tag="ohn", name="ohn")
                        nc.vector.tensor_tensor(out=ohn, in0=pav[:, :, 0:64],
                                                in1=_bcast_last(r4, 64),
                                                op=mybir.AluOpType.mult)
                        for tb in range(4):
                            nc.tensor.transpose(poht[hb:hb + 64, tb, :],
                                                ohn[:, tb, :], ident)
                    nc.vector.tensor_copy(
                        ohT_all[:, g, :].rearrange("p (a b) -> p a b", a=4), poht)

                # output projection for this t-chunk
                for jt in range(NTT):
                    ps_r = pmm.tile([128, 512], F32, tag="mm", name="psr")
                    for g in range(4):
                        nc.tensor.matmul(ps_r, wo_sb[:, g, jt * 128:(jt + 1) * 128],
                                         ohT_all[:, g, :], start=(g == 0),
                                         stop=(g == 3), skip_group_check=True)
                    rs = rsp.tile([128, 512], F32, tag="rs", name="rs")
                    nc.vector.tensor_copy(rs, ps_r)
                    eng = nc.sync if jt % 2 == 0 else nc.gpsimd
                    eng.dma_start(
                        out=res[jt * 128:(jt + 1) * 128, j * 512:(j + 1) * 512],
                        in_=rs)

            # Software pipeline with a one-quarter lag: projections for
            # quarter c are emitted before attention for chunk c-1 so the
            # shared psum pool rotation never serializes a chunk's attention
            # against the next quarter's projections.
            for c in range(NCH):
                proj_quarter(c)
                if c >= 1:
                    attention_chunk(c - 1)
            attention_chunk(NCH - 1)
    return nc


_NC_CACHE = {}


def _get_nc(causal: bool):
    if causal not in _NC_CACHE:
        _NC_CACHE[causal] = _build(causal)
    return _NC_CACHE[causal]


# ---------------------------------------------------------------------------
# Host wrapper
# ---------------------------------------------------------------------------

def kernel(x, cos, sin, mask, wq, wk, wv, wo):
    x = np.asarray(x, dtype=np.float32)
    cos = np.asarray(cos, dtype=np.float32)
    sin = np.asarray(sin, dtype=np.float32)
    mask = np.asarray(mask)
    wq = np.asarray(wq, dtype=np.float32)
    wk = np.asarray(wk, dtype=np.float32)
    wv = np.asarray(wv, dtype=np.float32)
    wo = np.asarray(wo, dtype=np.float32)

    m2 = mask[0, 0]
    tril = np.tril(np.ones((T, T), dtype=bool))
    if np.array_equal(m2, tril):
        causal = True
    elif m2.all():
        causal = False
    else:
        return _numpy_fallback(x, cos, sin, mask, wq, wk, wv, wo)

    _install_waitsplit()
    nc = _get_nc(causal)

    cexp = np.concatenate([cos, cos], axis=1).astype(np.float32)
    sexp = np.concatenate([-sin, sin], axis=1).astype(np.float32)

    in_maps = []
    for c in range(NCORES):
        b, jg = c // 4, c % 4
        heads = []
        for g in range(4):
            heads.append(8 * jg + g)
            heads.append(8 * jg + 4 + g)
        wq_rows = np.concatenate([wq[h * HD:(h + 1) * HD, :] for h in heads], axis=0)
        wo_cols = np.concatenate([wo[:, h * HD:(h + 1) * HD].T for h in heads], axis=0)
        kv = [2 * jg, 2 * jg + 1]
        wk_rows = np.concatenate([wk[k * HD:(k + 1) * HD, :] for k in kv], axis=0)
        wv_rows = np.concatenate([wv[k * HD:(k + 1) * HD, :] for k in kv], axis=0)
        wkv_cols = np.concatenate([wk_rows.T, wv_rows.T], axis=1)  # [D, 256]
        in_maps.append({
            "xt": np.ascontiguousarray(x[b].T).astype(NBF),
            "wq": np.ascontiguousarray(wq_rows.T).astype(NBF),
            "wkv": np.ascontiguousarray(wkv_cols).astype(NBF),
            "wo": np.ascontiguousarray(wo_cols).astype(NBF),
            "cexp": cexp,
            "sexp": sexp,
        })

    r = run_bass_kernel_spmd(nc, in_maps, core_ids=list(range(NCORES)))

    out = np.zeros((2, T, D), dtype=np.float32)
    for c in range(NCORES):
        out[c // 4] += r.results[c]["res"].T
    return out


def _numpy_fallback(x, cos, sin, mask, wq, wk, wv, wo):
    B = x.shape[0]
    NH, NKV = 32, 8
    q = (x @ wq.T).reshape(B, T, NH, HD).transpose(0, 2, 1, 3)
    k = (x @ wk.T).reshape(B, T, NKV, HD).transpose(0, 2, 1, 3)
    v = (x @ wv.T).reshape(B, T, NKV, HD).transpose(0, 2, 1, 3)

    def rope_np(t4):
        c = cos[None, None]
        s = sin[None, None]
        t1, t2 = t4[..., :32], t4[..., 32:]
        return np.concatenate([t1 * c - t2 * s, t2 * c + t1 * s], axis=-1)

    q, k = rope_np(q), rope_np(k)
    k = np.repeat(k, 4, axis=1)
    v = np.repeat(v, 4, axis=1)
    att = np.einsum("bhtd,bhsd->bhts", q, k) / np.sqrt(HD)
    att = np.where(mask, att, -np.inf)
    att = att - att.max(axis=-1, keepdims=True)
    p = np.exp(att)
    p /= p.sum(axis=-1, keepdims=True)
    o = np.einsum("bhts,bhsd->bhtd", p, v)
    o = o.transpose(0, 2, 1, 3).reshape(B, T, -1)
    return (o @ wo.T).astype(np.float32)


# revision 18
# speedup vs baseline: 1.3064x; 1.0039x over previous
"""GQA attention kernel for 8 Trainium2 cores (v2).

Problem: B=2, T=2048, D=2048, 32 q-heads, 8 kv-heads, head_dim=64, causal.

Sharding: core c = (b, jg) with b = c//4, jg = c%4. Each core handles batch b,
kv-heads {2jg, 2jg+1} and q-heads {8jg..8jg+7} (data parallel on B, tensor
parallel on heads; wq/wk/wv column-sharded, wo row-sharded). Each core returns
a partial output projection resT [D, T]; the host sums the 4 partials per
batch and transposes.

v2 design changes vs the 406us baseline:
 - input DMAs spread over the SP/ACT/POOL queues with wkv first so the first
   projection matmul starts at ~14us instead of ~44us.
 - k/q [t,f]->[f,t] transposes moved off the PE onto the DMA XBAR
   (dma_start_transpose), eliminating PE transpose+copy traffic.
 - AV matmul transposed: out[t(128), f(65)] accumulates with P-tiles as the
   stationary operand and [V|1] as the 65-row moving operand (65 rows vs 512
   rows per s-tile on the PE). The 65th column of the accumulator is the
   softmax denominator, so no separate reduction and no DRAM round-trip
   broadcast: reciprocal via ACT Ln/Exp on a [128,4,1] column, applied as a
   free-dim step-0 broadcast multiply.
 - PSUM accumulators for the 4 t-blocks share one bank; groups cannot use
   start=True (2KB zero-region granularity), so the bank is DVE-memset to 0
   and all AV matmuls accumulate with start=False.
 - causal diagonal computed at 128-column granularity (only the lower
   trapezoid), saving ~37% of diagonal scores/exp/AV work; only the true
   diagonal 128x128 subtiles get the multiplicative tril mask (on gpsimd).
 - output-projection results staged through SBUF (DMA cannot read PSUM) and
   streamed out per 128-row tile on the idle SP/POOL queues.
"""

import os
import sys

sys.path.insert(0, "/opt/trn_rl_repo")

import json

import numpy as np
import ml_dtypes

import concourse.bass as bass
import concourse.mybir as mybir
from concourse.tile import TileContext
from concourse.masks import make_identity
from concourse.bass_utils import run_bass_kernel_spmd

BF16 = mybir.dt.bfloat16
FP8 = mybir.dt.float8e4
F32 = mybir.dt.float32

T = 2048
D = 2048
HD = 64
NCORES = 8
KT = D // 128           # 16 contraction tiles
NTT = T // 128          # 16 time tiles
NCH = T // 512          # 4 chunks
NBF = ml_dtypes.bfloat16

# ---------------------------------------------------------------------------
# BIR post-pass: split multi-wait instructions into single-wait
# EventSemaphore carriers (the walrus build here allows one wait per inst).
# ---------------------------------------------------------------------------
_ws_ctr = [0]


def _split_waits_bytes(bir: bytes) -> bytes:
    d = json.loads(bir)
    for f in d.get("functions", []):
        for bb in f.get("blocks", []):
            out = []
            for inst in bb.get("instructions", []):
                si = inst.get("sync_info")
                waits = (si or {}).get("on_wait") or []
                if len(waits) > 1:
                    for w in waits[:-1]:
                        _ws_ctr[0] += 1
                        out.append({
                            "debug": inst.get("debug", 0),
                            "engine": inst["engine"],
                            "ins": [],
                            "name": f"WS-{_ws_ctr[0]}",
                            "opcode": "EventSemaphore",
                            "outs": [],
                            "sync_info": {"on_update": [], "on_wait": [w]},
                        })
                    si["on_wait"] = [waits[-1]]
                out.append(inst)
            bb["instructions"] = out
    return json.dumps(d).encode()


def _install_waitsplit():
    import concourse.bass2jax as b2j

    if getattr(b2j, "_waitsplit_installed", False):
        return
    orig = b2j._decompress_ant_bir
    b2j._decompress_ant_bir = lambda s: _split_waits_bytes(orig(s))
    b2j._waitsplit_installed = True


# ---------------------------------------------------------------------------
# Device program
# ---------------------------------------------------------------------------

def _bcast(ap2d, nh):
    """Insert a step-0 head dim into a [p, w] AP -> [p, nh, w]."""
    return bass.AP(tensor=ap2d.tensor, offset=ap2d.offset,
                   ap=[ap2d.ap[0], [0, nh], ap2d.ap[1]])


def _bcast_last(ap3d, w):
    """Append a step-0 last dim to a [p, n, 1] AP -> [p, n, w]."""
    return bass.AP(tensor=ap3d.tensor, offset=ap3d.offset,
                   ap=[ap3d.ap[0], ap3d.ap[1], [0, w]])


def _build(causal: bool):
    nc = bass.Bass()
    xt = nc.dram_tensor("xt", [D, T], BF16, kind="ExternalInput")
    wq = nc.dram_tensor("wq", [D, 512], BF16, kind="ExternalInput")
    wkv = nc.dram_tensor("wkv", [D, 256], BF16, kind="ExternalInput")
    wo = nc.dram_tensor("wo", [512, D], BF16, kind="ExternalInput")
    cexp = nc.dram_tensor("cexp", [T, 64], F32, kind="ExternalInput")
    sexp = nc.dram_tensor("sexp", [T, 64], F32, kind="ExternalInput")
    res = nc.dram_tensor("res", [D, T], F32, kind="ExternalOutput")

    with TileContext(nc) as tc:
        with (
            tc.tile_pool(name="const", bufs=1) as const,
            tc.tile_pool(name="big", bufs=1) as big,
            tc.tile_pool(name="ropew", bufs=3) as ropew,
            tc.tile_pool(name="qtfp", bufs=3) as qtfp,
            tc.tile_pool(name="ptp", bufs=4) as ptp,
            tc.tile_pool(name="ohp", bufs=3) as ohp,
            tc.tile_pool(name="rp", bufs=3) as rp,
            tc.tile_pool(name="ohtp", bufs=6) as ohtp,
            tc.tile_pool(name="rsp", bufs=3) as rsp,
            tc.tile_pool(name="pmm", bufs=2, space="PSUM") as pmm,
            tc.tile_pool(name="psc", bufs=2, space="PSUM") as psc,
            tc.tile_pool(name="pav", bufs=2, space="PSUM") as pavp,
        ):
            # ---------------- constants ----------------
            ident = const.tile([128, 128], BF16)
            make_identity(nc, ident)

            mtri = None
            if causal:
                # keep col >= row (upper triangle incl diagonal) of a
                # [s_local, t_local] 128x128 tile
                mtri = const.tile([128, 128], BF16)
                nc.vector.memset(mtri, 1.0)
                nc.gpsimd.affine_select(
                    out=mtri, in_=mtri, pattern=[[1, 128]], base=0,
                    channel_multiplier=-1, compare_op=mybir.AluOpType.is_ge,
                    fill=0.0)

            # ---------------- weights + x loads, spread over 3 queues ------
            wkv_sb = big.tile([128, KT, 256], BF16)
            nc.sync.dma_start(out=wkv_sb, in_=wkv.rearrange("(n p) c -> p n c", p=128))

            cexp_sb = const.tile([128, NTT, 64], F32)
            sexp_sb = const.tile([128, NTT, 64], F32)
            nc.scalar.dma_start(out=cexp_sb, in_=cexp.rearrange("(n p) c -> p n c", p=128))
            nc.scalar.dma_start(out=sexp_sb, in_=sexp.rearrange("(n p) c -> p n c", p=128))

            wq_sb = big.tile([128, KT, 512], BF16)
            nc.gpsimd.dma_start(out=wq_sb, in_=wq.rearrange("(n p) c -> p n c", p=128))

            xt_sb = big.tile([128, KT, T], BF16)
            xq = [nc.sync, nc.scalar, nc.gpsimd]
            for kt in range(KT):
                xq[kt % 3].dma_start(out=xt_sb[:, kt, :],
                                     in_=xt[kt * 128:(kt + 1) * 128, :])

            wo_sb = big.tile([128, 4, D], BF16)
            nc.scalar.dma_start(out=wo_sb, in_=wo.rearrange("(g p) d -> p g d", p=128))

            kT_sb = big.tile([128, NTT, 128], BF16)
            qT_all = big.tile([128, NTT, 512], BF16)
            # [kv0 | 1 | kv1 | 1]: v features for both kv heads plus the
            # all-ones denominator columns, one copy per s-tile
            vp = big.tile([128, NTT, 2, 65], BF16)
            nc.vector.memset(vp[:, :, :, 64:65], 1.0)

            def rope(ps3, out_bf, ti, nh):
                """RoPE in [t, f] layout. ps3: PSUM [128, nh, 64] f32 view ->
                out_bf: SBUF [128, nh*64] bf16. One DVE op to drain PSUM
                fast; the arithmetic runs on gpsimd from SBUF."""
                o3 = out_bf.rearrange("p (h k) -> p h k", h=nh)
                tmp = ropew.tile([128, nh, 64], F32, tag="ropeT", name="rt")
                nc.vector.tensor_copy(tmp, ps3)
                a = ropew.tile([128, nh, 64], F32, tag="ropeA", name="ra")
                bt = ropew.tile([128, nh, 64], F32, tag="ropeB", name="rb")
                cb = _bcast(cexp_sb[:, ti, :], nh)
                nc.gpsimd.tensor_tensor(out=a, in0=tmp, in1=cb, op=mybir.AluOpType.mult)
                sb_lo = _bcast(sexp_sb[:, ti, 0:32], nh)
                sb_hi = _bcast(sexp_sb[:, ti, 32:64], nh)
                nc.gpsimd.tensor_tensor(out=bt[:, :, 0:32], in0=tmp[:, :, 32:64],
                                        in1=sb_lo, op=mybir.AluOpType.mult)
                nc.gpsimd.tensor_tensor(out=bt[:, :, 32:64], in0=tmp[:, :, 0:32],
                                        in1=sb_hi, op=mybir.AluOpType.mult)
                nc.gpsimd.tensor_tensor(out=o3, in0=a, in1=bt, op=mybir.AluOpType.add)

            # ------- projections for one quarter of the time axis ----------
            def proj_quarter(c):
                for st in range(4 * c, 4 * c + 4):
                    ps_kv = pmm.tile([128, 256], F32, tag="mm", name="pskv")
                    for kt in range(KT):
                        nc.tensor.matmul(ps_kv, xt_sb[:, kt, st * 128:(st + 1) * 128],
                                         wkv_sb[:, kt, :], start=(kt == 0),
                                         stop=(kt == KT - 1))
                    ktf = qtfp.tile([128, 128], BF16, tag="qtf", name="ktf")
                    rope(ps_kv[:, 0:128].rearrange("p (h k) -> p h k", h=2),
                         ktf, st, 2)
                    nc.sync.dma_start_transpose(kT_sb[:, st, :], ktf)
                    nc.vector.tensor_copy(
                        vp[:, st, :, 0:64],
                        ps_kv[:, 128:256].rearrange("p (h k) -> p h k", h=2))
                for ti in range(4 * c, 4 * c + 4):
                    ps_q = pmm.tile([128, 512], F32, tag="mm", name="psq")
                    for kt in range(KT):
                        nc.tensor.matmul(ps_q, xt_sb[:, kt, ti * 128:(ti + 1) * 128],
                                         wq_sb[:, kt, :], start=(kt == 0),
                                         stop=(kt == KT - 1))
                    qtf = qtfp.tile([128, 512], BF16, tag="qtf", name="qtf")
                    rope(ps_q.rearrange("p (h k) -> p h k", h=8), qtf, ti, 8)
                    tt = ti % 4
                    for g in range(4):
                        nc.sync.dma_start_transpose(
                            qT_all[:, 4 * c + g, tt * 128:(tt + 1) * 128],
                            qtf[:, g * 128:(g + 1) * 128])

            # ------- attention + output projection for one 512-wide chunk --
            def attention_chunk(j):
                ohT_all = ohtp.tile([128, 4, 512], BF16, tag="oht", name="oht")
                for g in range(4):
                    qT = qT_all[:, 4 * j + g, :]
                    poht = pmm.tile([128, 4, 128], BF16, tag="mm", name="poht")
                    for half in (0, 1):
                        hb = 64 * half
                        pav = pavp.tile([128, 4, 128], F32, tag="av", name="pav")
                        nc.vector.memset(pav[:, :, 0:65], 0.0)

                        def av(ptile, si, tb0):
                            for tb in range(tb0, 4):
                                nc.tensor.matmul(
                                    pav[:, tb, 0:65],
                                    ptile[:, (tb - tb0) * 128:(tb - tb0 + 1) * 128],
                                    vp[:, si, half, :], start=False, stop=False,
                                    skip_group_check=True)

                        ns_off = 4 * j if causal else NTT
                        for sp in range(0, ns_off, 2):
                            ps2 = psc.tile([128, 2, 512], F32, tag="sc", name="pss")
                            for u in (0, 1):
                                nc.tensor.matmul(
                                    ps2[:, u, :], kT_sb[hb:hb + 64, sp + u, :],
                                    qT[hb:hb + 64, :],
                                    start=True, stop=True, skip_group_check=True)
                            pt2 = ptp.tile([128, 2, 512], BF16, tag="pt", name="pt")
                            nc.scalar.activation(out=pt2, in_=ps2,
                                                 func=mybir.ActivationFunctionType.Exp,
                                                 scale=0.125)
                            for u in (0, 1):
                                av(pt2[:, u, :], sp + u, 0)

                        if causal:
                            s0 = 4 * j
                            psd1 = psc.tile([128, 2, 512], F32, tag="sc", name="psd1")
                            psd2 = psc.tile([128, 2, 512], F32, tag="sc", name="psd2")
                            nc.tensor.matmul(psd1[:, 0, :],
                                             kT_sb[hb:hb + 64, s0, :],
                                             qT[hb:hb + 64, :],
                                             start=True, stop=True,
                                             skip_group_check=True)
                            nc.tensor.matmul(psd1[:, 1, 0:384],
                                             kT_sb[hb:hb + 64, s0 + 1, :],
                                             qT[hb:hb + 64, 128:512],
                                             start=True, stop=True,
                                             skip_group_check=True)
                            nc.tensor.matmul(psd1[:, 1, 384:512],
                                             kT_sb[hb:hb + 64, s0 + 3, :],
                                             qT[hb:hb + 64, 384:512],
                                             start=True, stop=True,
                                             skip_group_check=True)
                            nc.tensor.matmul(psd2[:, 0, 0:256],
                                             kT_sb[hb:hb + 64, s0 + 2, :],
                                             qT[hb:hb + 64, 256:512],
                                             start=True, stop=True,
                                             skip_group_check=True)
                            pd1 = ptp.tile([128, 2, 512], BF16, tag="pt", name="pd1")
                            nc.scalar.activation(out=pd1, in_=psd1,
                                                 func=mybir.ActivationFunctionType.Exp,
                                                 scale=0.125)
                            pd2 = ptp.tile([128, 2, 512], BF16, tag="pt", name="pd2")
                            nc.scalar.activation(out=pd2[:, 0, 0:256],
                                                 in_=psd2[:, 0, 0:256],
                                                 func=mybir.ActivationFunctionType.Exp,
                                                 scale=0.125)
                            for msk in (pd1[:, 0, 0:128], pd1[:, 1, 0:128],
                                        pd1[:, 1, 384:512], pd2[:, 0, 0:128]):
                                nc.gpsimd.tensor_tensor(out=msk, in0=msk, in1=mtri,
                                                        op=mybir.AluOpType.mult)
                            av(pd1[:, 0, :], s0, 0)
                            av(pd1[:, 1, 0:384], s0 + 1, 1)
                            av(pd2[:, 0, 0:256], s0 + 2, 2)
                            av(pd1[:, 1, 384:512], s0 + 3, 3)

                        # softmax denominator -> reciprocal -> normalize
                        r4 = rp.tile([128, 4, 1], F32, tag="r4", name="r4")
                        nc.vector.reciprocal(out=r4, in_=pav[:, :, 64:65])
                        ohn = ohp.tile([128, 4, 64], BF16, tag="ohn", name="ohn")
                        nc.vector.tensor_tensor(out=ohn, in0=pav[:, :, 0:64],
                                                in1=_bcast_last(r4, 64),
                                                op=mybir.AluOpType.mult)
                        for tb in range(4):
                            nc.tensor.transpose(poht[hb:hb + 64, tb, :],
                                                ohn[:, tb, :], ident)
                    nc.vector.tensor_copy(
                        ohT_all[:, g, :].rearrange("p (a b) -> p a b", a=4), poht)

                # output projection for this t-chunk
                for jt in range(NTT):
                    ps_r = pmm.tile([128, 512], F32, tag="mm", name="psr")
                    for g in range(4):
                        nc.tensor.matmul(ps_r, wo_sb[:, g, jt * 128:(jt + 1) * 128],
                                         ohT_all[:, g, :], start=(g == 0),
                                         stop=(g == 3), skip_group_check=True)
                    rs = rsp.tile([128, 512], F32, tag="rs", name="rs")
                    nc.vector.tensor_copy(rs, ps_r)
                    eng = nc.sync if jt % 2 == 0 else nc.gpsimd
                    eng.dma_start(
                        out=res[jt * 128:(jt + 1) * 128, j * 512:(j + 1) * 512],
                        in_=rs)

            # Software pipeline with a one-quarter lag: projections for
            # quarter c are emitted before attention for chunk c-1 so the
            # shared psum pool rotation never serializes a chunk's attention
            # against the next quarter's projections.
            for c in range(NCH):
                proj_quarter(c)
                if c >= 1:
                    attention_chunk(c - 1)
            attention_chunk(NCH - 1)
    return nc


_NC_CACHE = {}


def _get_nc(causal: bool):
    if causal not in _NC_CACHE:
        _NC_CACHE[causal] = _build(causal)
    return _NC_CACHE[causal]


# ---------------------------------------------------------------------------
# Host wrapper
# ---------------------------------------------------------------------------

def kernel(x, cos, sin, mask, wq, wk, wv, wo):
    x = np.asarray(x, dtype=np.float32)
    cos = np.asarray(cos, dtype=np.float32)
    sin = np.asarray(sin, dtype=np.float32)
    mask = np.asarray(mask)
    wq = np.asarray(wq, dtype=np.float32)
    wk = np.asarray(wk, dtype=np.float32)
    wv = np.asarray(wv, dtype=np.float32)
    wo = np.asarray(wo, dtype=np.float32)

    m2 = mask[0, 0]
    tril = np.tril(np.ones((T, T), dtype=bool))
    if np.array_equal(m2, tril):
        causal = True
    elif m2.all():
        causal = False
    else:
        return _numpy_fallback(x, cos, sin, mask, wq, wk, wv, wo)

    _install_waitsplit()
    nc = _get_nc(causal)

    cexp = np.concatenate([cos, cos], axis=1).astype(np.float32)
    sexp = np.concatenate([-sin, sin], axis=1).astype(np.float32)

    in_maps = []
    for c in range(NCORES):
        b, jg = c // 4, c % 4
        heads = []
        for g in range(4):
            heads.append(8 * jg + g)
            heads.append(8 * jg + 4 + g)
        wq_rows = np.concatenate([wq[h * HD:(h + 1) * HD, :] for h in heads], axis=0)
        wo_cols = np.concatenate([wo[:, h * HD:(h + 1) * HD].T for h in heads], axis=0)
        kv = [2 * jg, 2 * jg + 1]
        wk_rows = np.concatenate([wk[k * HD:(k + 1) * HD, :] for k in kv], axis=0)
        wv_rows = np.concatenate([wv[k * HD:(k + 1) * HD, :] for k in kv], axis=0)
        wkv_cols = np.concatenate([wk_rows.T, wv_rows.T], axis=1)  # [D, 256]
        in_maps.append({
            "xt": np.ascontiguousarray(x[b].T).astype(NBF),
            "wq": np.ascontiguousarray(wq_rows.T).astype(NBF),
            "wkv": np.ascontiguousarray(wkv_cols).astype(NBF),
            "wo": np.ascontiguousarray(wo_cols).astype(NBF),
            "cexp": cexp,
            "sexp": sexp,
        })

    r = run_bass_kernel_spmd(nc, in_maps, core_ids=list(range(NCORES)))

    out = np.zeros((2, T, D), dtype=np.float32)
    for c in range(NCORES):
        out[c // 4] += r.results[c]["res"].T
    return out


def _numpy_fallback(x, cos, sin, mask, wq, wk, wv, wo):
    B = x.shape[0]
    NH, NKV = 32, 8
    q = (x @ wq.T).reshape(B, T, NH, HD).transpose(0, 2, 1, 3)
    k = (x @ wk.T).reshape(B, T, NKV, HD).transpose(0, 2, 1, 3)
    v = (x @ wv.T).reshape(B, T, NKV, HD).transpose(0, 2, 1, 3)

    def rope_np(t4):
        c = cos[None, None]
        s = sin[None, None]
        t1, t2 = t4[..., :32], t4[..., 32:]
        return np.concatenate([t1 * c - t2 * s, t2 * c + t1 * s], axis=-1)

    q, k = rope_np(q), rope_np(k)
    k = np.repeat(k, 4, axis=1)
    v = np.repeat(v, 4, axis=1)
    att = np.einsum("bhtd,bhsd->bhts", q, k) / np.sqrt(HD)
    att = np.where(mask, att, -np.inf)
    att = att - att.max(axis=-1, keepdims=True)
    p = np.exp(att)
    p /= p.sum(axis=-1, keepdims=True)
    o = np.einsum("bhts,bhsd->bhtd", p, v)
    o = o.transpose(0, 2, 1, 3).reshape(B, T, -1)
    return (o @ wo.T).astype(np.float32)


# revision 19
# speedup vs baseline: 1.3300x; 1.0181x over previous
"""GQA attention kernel for 8 Trainium2 cores (v2).

Problem: B=2, T=2048, D=2048, 32 q-heads, 8 kv-heads, head_dim=64, causal.

Sharding: core c = (b, jg) with b = c//4, jg = c%4. Each core handles batch b,
kv-heads {2jg, 2jg+1} and q-heads {8jg..8jg+7} (data parallel on B, tensor
parallel on heads; wq/wk/wv column-sharded, wo row-sharded). Each core returns
a partial output projection resT [D, T]; the host sums the 4 partials per
batch and transposes.

v2 design changes vs the 406us baseline:
 - input DMAs spread over the SP/ACT/POOL queues with wkv first so the first
   projection matmul starts at ~14us instead of ~44us.
 - k/q [t,f]->[f,t] transposes moved off the PE onto the DMA XBAR
   (dma_start_transpose), eliminating PE transpose+copy traffic.
 - AV matmul transposed: out[t(128), f(65)] accumulates with P-tiles as the
   stationary operand and [V|1] as the 65-row moving operand (65 rows vs 512
   rows per s-tile on the PE). The 65th column of the accumulator is the
   softmax denominator, so no separate reduction and no DRAM round-trip
   broadcast: reciprocal via ACT Ln/Exp on a [128,4,1] column, applied as a
   free-dim step-0 broadcast multiply.
 - PSUM accumulators for the 4 t-blocks share one bank; groups cannot use
   start=True (2KB zero-region granularity), so the bank is DVE-memset to 0
   and all AV matmuls accumulate with start=False.
 - causal diagonal computed at 128-column granularity (only the lower
   trapezoid), saving ~37% of diagonal scores/exp/AV work; only the true
   diagonal 128x128 subtiles get the multiplicative tril mask (on gpsimd).
 - output-projection results staged through SBUF (DMA cannot read PSUM) and
   streamed out per 128-row tile on the idle SP/POOL queues.
"""

import os
import sys

sys.path.insert(0, "/opt/trn_rl_repo")

import json

import numpy as np
import ml_dtypes

import concourse.bass as bass
import concourse.mybir as mybir
from concourse.tile import TileContext
from concourse.masks import make_identity
from concourse.bass_utils import run_bass_kernel_spmd

BF16 = mybir.dt.bfloat16
FP8 = mybir.dt.float8e4
F32 = mybir.dt.float32

T = 2048
D = 2048
HD = 64
NCORES = 8
KT = D // 128           # 16 contraction tiles
NTT = T // 128          # 16 time tiles
NCH = T // 512          # 4 chunks
NBF = ml_dtypes.bfloat16

# ---------------------------------------------------------------------------
# BIR post-pass: split multi-wait instructions into single-wait
# EventSemaphore carriers (the walrus build here allows one wait per inst).
# ---------------------------------------------------------------------------
_ws_ctr = [0]


def _split_waits_bytes(bir: bytes) -> bytes:
    d = json.loads(bir)
    for f in d.get("functions", []):
        for bb in f.get("blocks", []):
            out = []
            for inst in bb.get("instructions", []):
                si = inst.get("sync_info")
                waits = (si or {}).get("on_wait") or []
                if len(waits) > 1:
                    for w in waits[:-1]:
                        _ws_ctr[0] += 1
                        out.append({
                            "debug": inst.get("debug", 0),
                            "engine": inst["engine"],
                            "ins": [],
                            "name": f"WS-{_ws_ctr[0]}",
                            "opcode": "EventSemaphore",
                            "outs": [],
                            "sync_info": {"on_update": [], "on_wait": [w]},
                        })
                    si["on_wait"] = [waits[-1]]
                out.append(inst)
            bb["instructions"] = out
    return json.dumps(d).encode()


def _install_waitsplit():
    import concourse.bass2jax as b2j

    if getattr(b2j, "_waitsplit_installed", False):
        return
    orig = b2j._decompress_ant_bir
    b2j._decompress_ant_bir = lambda s: _split_waits_bytes(orig(s))
    b2j._waitsplit_installed = True


# ---------------------------------------------------------------------------
# Device program
# ---------------------------------------------------------------------------

def _bcast(ap2d, nh):
    """Insert a step-0 head dim into a [p, w] AP -> [p, nh, w]."""
    return bass.AP(tensor=ap2d.tensor, offset=ap2d.offset,
                   ap=[ap2d.ap[0], [0, nh], ap2d.ap[1]])


def _bcast_last(ap3d, w):
    """Append a step-0 last dim to a [p, n, 1] AP -> [p, n, w]."""
    return bass.AP(tensor=ap3d.tensor, offset=ap3d.offset,
                   ap=[ap3d.ap[0], ap3d.ap[1], [0, w]])


def _build(causal: bool):
    nc = bass.Bass()
    xt = nc.dram_tensor("xt", [D, T], BF16, kind="ExternalInput")
    wq = nc.dram_tensor("wq", [D, 512], BF16, kind="ExternalInput")
    wkv = nc.dram_tensor("wkv", [D, 256], BF16, kind="ExternalInput")
    wo = nc.dram_tensor("wo", [512, D], BF16, kind="ExternalInput")
    cexp = nc.dram_tensor("cexp", [T, 64], F32, kind="ExternalInput")
    sexp = nc.dram_tensor("sexp", [T, 64], F32, kind="ExternalInput")
    res = nc.dram_tensor("res", [D, T], F32, kind="ExternalOutput")

    with TileContext(nc) as tc:
        with (
            tc.tile_pool(name="const", bufs=1) as const,
            tc.tile_pool(name="big", bufs=1) as big,
            tc.tile_pool(name="ropew", bufs=3) as ropew,
            tc.tile_pool(name="qtfp", bufs=3) as qtfp,
            tc.tile_pool(name="ptp", bufs=4) as ptp,
            tc.tile_pool(name="ohp", bufs=3) as ohp,
            tc.tile_pool(name="rp", bufs=3) as rp,
            tc.tile_pool(name="ohtp", bufs=6) as ohtp,
            tc.tile_pool(name="rsp", bufs=3) as rsp,
            tc.tile_pool(name="pmm", bufs=2, space="PSUM") as pmm,
            tc.tile_pool(name="psc", bufs=2, space="PSUM") as psc,
            tc.tile_pool(name="pav", bufs=2, space="PSUM") as pavp,
        ):
            # ---------------- constants ----------------
            ident = const.tile([128, 128], BF16)
            make_identity(nc, ident)

            mtri = None
            if causal:
                # keep col >= row (upper triangle incl diagonal) of a
                # [s_local, t_local] 128x128 tile
                mtri = const.tile([128, 128], BF16)
                nc.vector.memset(mtri, 1.0)
                nc.gpsimd.affine_select(
                    out=mtri, in_=mtri, pattern=[[1, 128]], base=0,
                    channel_multiplier=-1, compare_op=mybir.AluOpType.is_ge,
                    fill=0.0)

            # ---------------- weights + x loads, spread over 3 queues ------
            wkv_sb = big.tile([128, KT, 256], BF16)
            nc.sync.dma_start(out=wkv_sb, in_=wkv.rearrange("(n p) c -> p n c", p=128))

            cexp_sb = const.tile([128, NTT, 64], F32)
            sexp_sb = const.tile([128, NTT, 64], F32)
            nc.scalar.dma_start(out=cexp_sb, in_=cexp.rearrange("(n p) c -> p n c", p=128))
            nc.scalar.dma_start(out=sexp_sb, in_=sexp.rearrange("(n p) c -> p n c", p=128))

            wq_sb = big.tile([128, KT, 512], BF16)
            nc.gpsimd.dma_start(out=wq_sb, in_=wq.rearrange("(n p) c -> p n c", p=128))

            xt_sb = big.tile([128, KT, T], BF16)
            xq = [nc.sync, nc.scalar, nc.gpsimd]

            wo_sb = big.tile([128, 4, D], BF16)

            kT_sb = big.tile([128, NTT, 128], BF16)
            qT_all = big.tile([128, NTT, 512], BF16)
            # [kv0 | 1 | kv1 | 1]: v features for both kv heads plus the
            # all-ones denominator columns, one copy per s-tile
            vp = big.tile([128, NTT, 2, 65], BF16)
            nc.vector.memset(vp[:, :, :, 64:65], 1.0)

            def rope(ps3, out_bf, ti, nh):
                """RoPE in [t, f] layout. ps3: PSUM [128, nh, 64] f32 view ->
                out_bf: SBUF [128, nh*64] bf16. One DVE op to drain PSUM
                fast; the arithmetic runs on gpsimd from SBUF."""
                o3 = out_bf.rearrange("p (h k) -> p h k", h=nh)
                tmp = ropew.tile([128, nh, 64], F32, tag="ropeT", name="rt")
                nc.vector.tensor_copy(tmp, ps3)
                a = ropew.tile([128, nh, 64], F32, tag="ropeA", name="ra")
                bt = ropew.tile([128, nh, 64], F32, tag="ropeB", name="rb")
                cb = _bcast(cexp_sb[:, ti, :], nh)
                nc.gpsimd.tensor_tensor(out=a, in0=tmp, in1=cb, op=mybir.AluOpType.mult)
                sb_lo = _bcast(sexp_sb[:, ti, 0:32], nh)
                sb_hi = _bcast(sexp_sb[:, ti, 32:64], nh)
                nc.gpsimd.tensor_tensor(out=bt[:, :, 0:32], in0=tmp[:, :, 32:64],
                                        in1=sb_lo, op=mybir.AluOpType.mult)
                nc.gpsimd.tensor_tensor(out=bt[:, :, 32:64], in0=tmp[:, :, 0:32],
                                        in1=sb_hi, op=mybir.AluOpType.mult)
                nc.gpsimd.tensor_tensor(out=o3, in0=a, in1=bt, op=mybir.AluOpType.add)

            # ------- projections for one quarter of the time axis ----------
            def proj_quarter(c):
                lo, hi = c * 512, (c + 1) * 512
                for kt in range(KT):
                    xq[kt % 3].dma_start(out=xt_sb[:, kt, lo:hi],
                                         in_=xt[kt * 128:(kt + 1) * 128, lo:hi])
                if c == 0:
                    # wo is first needed by chunk 0's output projection; load
                    # it behind the quarter-0 x slices, spread over all queues
                    for g in range(4):
                        xq[g % 3].dma_start(
                            out=wo_sb[:, g, :], in_=wo[g * 128:(g + 1) * 128, :])
                for st in range(4 * c, 4 * c + 4):
                    ps_kv = pmm.tile([128, 256], F32, tag="mm", name="pskv")
                    for kt in range(KT):
                        nc.tensor.matmul(ps_kv, xt_sb[:, kt, st * 128:(st + 1) * 128],
                                         wkv_sb[:, kt, :], start=(kt == 0),
                                         stop=(kt == KT - 1))
                    ktf = qtfp.tile([128, 128], BF16, tag="qtf", name="ktf")
                    rope(ps_kv[:, 0:128].rearrange("p (h k) -> p h k", h=2),
                         ktf, st, 2)
                    nc.sync.dma_start_transpose(kT_sb[:, st, :], ktf)
                    nc.vector.tensor_copy(
                        vp[:, st, :, 0:64],
                        ps_kv[:, 128:256].rearrange("p (h k) -> p h k", h=2))
                for ti in range(4 * c, 4 * c + 4):
                    ps_q = pmm.tile([128, 512], F32, tag="mm", name="psq")
                    for kt in range(KT):
                        nc.tensor.matmul(ps_q, xt_sb[:, kt, ti * 128:(ti + 1) * 128],
                                         wq_sb[:, kt, :], start=(kt == 0),
                                         stop=(kt == KT - 1))
                    qtf = qtfp.tile([128, 512], BF16, tag="qtf", name="qtf")
                    rope(ps_q.rearrange("p (h k) -> p h k", h=8), qtf, ti, 8)
                    tt = ti % 4
                    for g in range(4):
                        nc.sync.dma_start_transpose(
                            qT_all[:, 4 * c + g, tt * 128:(tt + 1) * 128],
                            qtf[:, g * 128:(g + 1) * 128])

            # ------- attention + output projection for one 512-wide chunk --
            def attention_chunk(j):
                ohT_all = ohtp.tile([128, 4, 512], BF16, tag="oht", name="oht")
                for g in range(4):
                    qT = qT_all[:, 4 * j + g, :]
                    poht = pmm.tile([128, 4, 128], BF16, tag="mm", name="poht")
                    for half in (0, 1):
                        hb = 64 * half
                        pav = pavp.tile([128, 4, 128], F32, tag="av", name="pav")
                        nc.vector.memset(pav[:, :, 0:65], 0.0)

                        def av(ptile, si, tb0):
                            for tb in range(tb0, 4):
                                nc.tensor.matmul(
                                    pav[:, tb, 0:65],
                                    ptile[:, (tb - tb0) * 128:(tb - tb0 + 1) * 128],
                                    vp[:, si, half, :], start=False, stop=False,
                                    skip_group_check=True)

                        ns_off = 4 * j if causal else NTT
                        for sp in range(0, ns_off, 2):
                            ps2 = psc.tile([128, 2, 512], F32, tag="sc", name="pss")
                            for u in (0, 1):
                                nc.tensor.matmul(
                                    ps2[:, u, :], kT_sb[hb:hb + 64, sp + u, :],
                                    qT[hb:hb + 64, :],
                                    start=True, stop=True, skip_group_check=True)
                            pt2 = ptp.tile([128, 2, 512], BF16, tag="pt", name="pt")
                            nc.scalar.activation(out=pt2, in_=ps2,
                                                 func=mybir.ActivationFunctionType.Exp,
                                                 scale=0.125)
                            for u in (0, 1):
                                av(pt2[:, u, :], sp + u, 0)

                        if causal:
                            s0 = 4 * j
                            psd1 = psc.tile([128, 2, 512], F32, tag="sc", name="psd1")
                            psd2 = psc.tile([128, 2, 512], F32, tag="sc", name="psd2")
                            nc.tensor.matmul(psd1[:, 0, :],
                                             kT_sb[hb:hb + 64, s0, :],
                                             qT[hb:hb + 64, :],
                                             start=True, stop=True,
                                             skip_group_check=True)
                            nc.tensor.matmul(psd1[:, 1, 0:384],
                                             kT_sb[hb:hb + 64, s0 + 1, :],
                                             qT[hb:hb + 64, 128:512],
                                             start=True, stop=True,
                                             skip_group_check=True)
                            nc.tensor.matmul(psd1[:, 1, 384:512],
                                             kT_sb[hb:hb + 64, s0 + 3, :],
                                             qT[hb:hb + 64, 384:512],
                                             start=True, stop=True,
                                             skip_group_check=True)
                            nc.tensor.matmul(psd2[:, 0, 0:256],
                                             kT_sb[hb:hb + 64, s0 + 2, :],
                                             qT[hb:hb + 64, 256:512],
                                             start=True, stop=True,
                                             skip_group_check=True)
                            pd1 = ptp.tile([128, 2, 512], BF16, tag="pt", name="pd1")
                            nc.scalar.activation(out=pd1, in_=psd1,
                                                 func=mybir.ActivationFunctionType.Exp,
                                                 scale=0.125)
                            pd2 = ptp.tile([128, 2, 512], BF16, tag="pt", name="pd2")
                            nc.scalar.activation(out=pd2[:, 0, 0:256],
                                                 in_=psd2[:, 0, 0:256],
                                                 func=mybir.ActivationFunctionType.Exp,
                                                 scale=0.125)
                            for msk in (pd1[:, 0, 0:128], pd1[:, 1, 0:128],
                                        pd1[:, 1, 384:512], pd2[:, 0, 0:128]):
                                nc.gpsimd.tensor_tensor(out=msk, in0=msk, in1=mtri,
                                                        op=mybir.AluOpType.mult)
                            av(pd1[:, 0, :], s0, 0)
                            av(pd1[:, 1, 0:384], s0 + 1, 1)
                            av(pd2[:, 0, 0:256], s0 + 2, 2)
                            av(pd1[:, 1, 384:512], s0 + 3, 3)

                        # softmax denominator -> reciprocal -> normalize
                        r4 = rp.tile([128, 4, 1], F32, tag="r4", name="r4")
                        nc.vector.reciprocal(out=r4, in_=pav[:, :, 64:65])
                        ohn = ohp.tile([128, 4, 64], BF16, tag="ohn", name="ohn")
                        nc.vector.tensor_tensor(out=ohn, in0=pav[:, :, 0:64],
                                                in1=_bcast_last(r4, 64),
                                                op=mybir.AluOpType.mult)
                        for tb in range(4):
                            nc.tensor.transpose(poht[hb:hb + 64, tb, :],
                                                ohn[:, tb, :], ident)
                    nc.vector.tensor_copy(
                        ohT_all[:, g, :].rearrange("p (a b) -> p a b", a=4), poht)

                # output projection for this t-chunk
                for jt in range(NTT):
                    ps_r = pmm.tile([128, 512], F32, tag="mm", name="psr")
                    for g in range(4):
                        nc.tensor.matmul(ps_r, wo_sb[:, g, jt * 128:(jt + 1) * 128],
                                         ohT_all[:, g, :], start=(g == 0),
                                         stop=(g == 3), skip_group_check=True)
                    rs = rsp.tile([128, 512], F32, tag="rs", name="rs")
                    nc.vector.tensor_copy(rs, ps_r)
                    eng = nc.sync if jt % 2 == 0 else nc.gpsimd
                    eng.dma_start(
                        out=res[jt * 128:(jt + 1) * 128, j * 512:(j + 1) * 512],
                        in_=rs)

            # Software pipeline with a one-quarter lag: projections for
            # quarter c are emitted before attention for chunk c-1 so the
            # shared psum pool rotation never serializes a chunk's attention
            # against the next quarter's projections.
            for c in range(NCH):
                proj_quarter(c)
                if c >= 1:
                    attention_chunk(c - 1)
            attention_chunk(NCH - 1)
    return nc


_NC_CACHE = {}


def _get_nc(causal: bool):
    if causal not in _NC_CACHE:
        _NC_CACHE[causal] = _build(causal)
    return _NC_CACHE[causal]


# ---------------------------------------------------------------------------
# Host wrapper
# ---------------------------------------------------------------------------

def kernel(x, cos, sin, mask, wq, wk, wv, wo):
    x = np.asarray(x, dtype=np.float32)
    cos = np.asarray(cos, dtype=np.float32)
    sin = np.asarray(sin, dtype=np.float32)
    mask = np.asarray(mask)
    wq = np.asarray(wq, dtype=np.float32)
    wk = np.asarray(wk, dtype=np.float32)
    wv = np.asarray(wv, dtype=np.float32)
    wo = np.asarray(wo, dtype=np.float32)

    m2 = mask[0, 0]
    tril = np.tril(np.ones((T, T), dtype=bool))
    if np.array_equal(m2, tril):
        causal = True
    elif m2.all():
        causal = False
    else:
        return _numpy_fallback(x, cos, sin, mask, wq, wk, wv, wo)

    _install_waitsplit()
    nc = _get_nc(causal)

    cexp = np.concatenate([cos, cos], axis=1).astype(np.float32)
    sexp = np.concatenate([-sin, sin], axis=1).astype(np.float32)

    in_maps = []
    for c in range(NCORES):
        b, jg = c // 4, c % 4
        heads = []
        for g in range(4):
            heads.append(8 * jg + g)
            heads.append(8 * jg + 4 + g)
        wq_rows = np.concatenate([wq[h * HD:(h + 1) * HD, :] for h in heads], axis=0)
        wo_cols = np.concatenate([wo[:, h * HD:(h + 1) * HD].T for h in heads], axis=0)
        kv = [2 * jg, 2 * jg + 1]
        wk_rows = np.concatenate([wk[k * HD:(k + 1) * HD, :] for k in kv], axis=0)
        wv_rows = np.concatenate([wv[k * HD:(k + 1) * HD, :] for k in kv], axis=0)
        wkv_cols = np.concatenate([wk_rows.T, wv_rows.T], axis=1)  # [D, 256]
        in_maps.append({
            "xt": np.ascontiguousarray(x[b].T).astype(NBF),
            "wq": np.ascontiguousarray(wq_rows.T).astype(NBF),
            "wkv": np.ascontiguousarray(wkv_cols).astype(NBF),
            "wo": np.ascontiguousarray(wo_cols).astype(NBF),
            "cexp": cexp,
            "sexp": sexp,
        })

    r = run_bass_kernel_spmd(nc, in_maps, core_ids=list(range(NCORES)))

    out = np.zeros((2, T, D), dtype=np.float32)
    for c in range(NCORES):
        out[c // 4] += r.results[c]["res"].T
    return out


def _numpy_fallback(x, cos, sin, mask, wq, wk, wv, wo):
    B = x.shape[0]
    NH, NKV = 32, 8
    q = (x @ wq.T).reshape(B, T, NH, HD).transpose(0, 2, 1, 3)
    k = (x @ wk.T).reshape(B, T, NKV, HD).transpose(0, 2, 1, 3)
    v = (x @ wv.T).reshape(B, T, NKV, HD).transpose(0, 2, 1, 3)

    def rope_np(t4):
        c = cos[None, None]
        s = sin[None, None]
        t1, t2 = t4[..., :32], t4[..., 32:]
        return np.concatenate([t1 * c - t2 * s, t2 * c + t1 * s], axis=-1)

    q, k = rope_np(q), rope_np(k)
    k = np.repeat(k, 4, axis=1)
    v = np.repeat(v, 4, axis=1)
    att = np.einsum("bhtd,bhsd->bhts", q, k) / np.sqrt(HD)
    att = np.where(mask, att, -np.inf)
    att = att - att.max(axis=-1, keepdims=True)
    p = np.exp(att)
    p /= p.sum(axis=-1, keepdims=True)
    o = np.einsum("bhts,bhsd->bhtd", p, v)
    o = o.transpose(0, 2, 1, 3).reshape(B, T, -1)
    return (o @ wo.T).astype(np.float32)


# revision 21
# speedup vs baseline: 1.3543x; 1.0182x over previous
"""GQA attention kernel for 8 Trainium2 cores (v2).

Problem: B=2, T=2048, D=2048, 32 q-heads, 8 kv-heads, head_dim=64, causal.

Sharding: core c = (b, jg) with b = c//4, jg = c%4. Each core handles batch b,
kv-heads {2jg, 2jg+1} and q-heads {8jg..8jg+7} (data parallel on B, tensor
parallel on heads; wq/wk/wv column-sharded, wo row-sharded). Each core returns
a partial output projection resT [D, T]; the host sums the 4 partials per
batch and transposes.

v2 design changes vs the 406us baseline:
 - input DMAs spread over the SP/ACT/POOL queues with wkv first so the first
   projection matmul starts at ~14us instead of ~44us.
 - k/q [t,f]->[f,t] transposes moved off the PE onto the DMA XBAR
   (dma_start_transpose), eliminating PE transpose+copy traffic.
 - AV matmul transposed: out[t(128), f(65)] accumulates with P-tiles as the
   stationary operand and [V|1] as the 65-row moving operand (65 rows vs 512
   rows per s-tile on the PE). The 65th column of the accumulator is the
   softmax denominator, so no separate reduction and no DRAM round-trip
   broadcast: reciprocal via ACT Ln/Exp on a [128,4,1] column, applied as a
   free-dim step-0 broadcast multiply.
 - PSUM accumulators for the 4 t-blocks share one bank; groups cannot use
   start=True (2KB zero-region granularity), so the bank is DVE-memset to 0
   and all AV matmuls accumulate with start=False.
 - causal diagonal computed at 128-column granularity (only the lower
   trapezoid), saving ~37% of diagonal scores/exp/AV work; only the true
   diagonal 128x128 subtiles get the multiplicative tril mask (on gpsimd).
 - output-projection results staged through SBUF (DMA cannot read PSUM) and
   streamed out per 128-row tile on the idle SP/POOL queues.
"""

import os
import sys

sys.path.insert(0, "/opt/trn_rl_repo")

import json

import numpy as np
import ml_dtypes

import concourse.bass as bass
import concourse.mybir as mybir
from concourse.tile import TileContext
from concourse.masks import make_identity
from concourse.bass_utils import run_bass_kernel_spmd

BF16 = mybir.dt.bfloat16
FP8 = mybir.dt.float8e4
F32 = mybir.dt.float32

T = 2048
D = 2048
HD = 64
NCORES = 8
KT = D // 128           # 16 contraction tiles
NTT = T // 128          # 16 time tiles
NCH = T // 512          # 4 chunks
NBF = ml_dtypes.bfloat16

# ---------------------------------------------------------------------------
# BIR post-pass: split multi-wait instructions into single-wait
# EventSemaphore carriers (the walrus build here allows one wait per inst).
# ---------------------------------------------------------------------------
_ws_ctr = [0]


def _split_waits_bytes(bir: bytes) -> bytes:
    d = json.loads(bir)
    for f in d.get("functions", []):
        for bb in f.get("blocks", []):
            out = []
            for inst in bb.get("instructions", []):
                si = inst.get("sync_info")
                waits = (si or {}).get("on_wait") or []
                if len(waits) > 1:
                    for w in waits[:-1]:
                        _ws_ctr[0] += 1
                        out.append({
                            "debug": inst.get("debug", 0),
                            "engine": inst["engine"],
                            "ins": [],
                            "name": f"WS-{_ws_ctr[0]}",
                            "opcode": "EventSemaphore",
                            "outs": [],
                            "sync_info": {"on_update": [], "on_wait": [w]},
                        })
                    si["on_wait"] = [waits[-1]]
                out.append(inst)
            bb["instructions"] = out
    return json.dumps(d).encode()


def _install_waitsplit():
    import concourse.bass2jax as b2j

    if getattr(b2j, "_waitsplit_installed", False):
        return
    orig = b2j._decompress_ant_bir
    b2j._decompress_ant_bir = lambda s: _split_waits_bytes(orig(s))
    b2j._waitsplit_installed = True


# ---------------------------------------------------------------------------
# Device program
# ---------------------------------------------------------------------------

def _bcast(ap2d, nh):
    """Insert a step-0 head dim into a [p, w] AP -> [p, nh, w]."""
    return bass.AP(tensor=ap2d.tensor, offset=ap2d.offset,
                   ap=[ap2d.ap[0], [0, nh], ap2d.ap[1]])


def _bcast_last(ap3d, w):
    """Append a step-0 last dim to a [p, n, 1] AP -> [p, n, w]."""
    return bass.AP(tensor=ap3d.tensor, offset=ap3d.offset,
                   ap=[ap3d.ap[0], ap3d.ap[1], [0, w]])


def _build(causal: bool):
    nc = bass.Bass()
    xt = nc.dram_tensor("xt", [D, T], BF16, kind="ExternalInput")
    wq = nc.dram_tensor("wq", [D, 512], BF16, kind="ExternalInput")
    wkv = nc.dram_tensor("wkv", [D, 256], BF16, kind="ExternalInput")
    wo = nc.dram_tensor("wo", [512, D], BF16, kind="ExternalInput")
    cexp = nc.dram_tensor("cexp", [T, 64], F32, kind="ExternalInput")
    sexp = nc.dram_tensor("sexp", [T, 64], F32, kind="ExternalInput")
    res = nc.dram_tensor("res", [D, T], F32, kind="ExternalOutput")

    with TileContext(nc) as tc:
        with (
            tc.tile_pool(name="const", bufs=1) as const,
            tc.tile_pool(name="big", bufs=1) as big,
            tc.tile_pool(name="ropew", bufs=3) as ropew,
            tc.tile_pool(name="qtfp", bufs=3) as qtfp,
            tc.tile_pool(name="ptp", bufs=4) as ptp,
            tc.tile_pool(name="ohp", bufs=3) as ohp,
            tc.tile_pool(name="rp", bufs=3) as rp,
            tc.tile_pool(name="ohtp", bufs=6) as ohtp,
            tc.tile_pool(name="rsp", bufs=3) as rsp,
            tc.tile_pool(name="pmm", bufs=2, space="PSUM") as pmm,
            tc.tile_pool(name="psc", bufs=2, space="PSUM") as psc,
            tc.tile_pool(name="pav", bufs=2, space="PSUM") as pavp,
        ):
            # ---------------- constants ----------------
            ident = const.tile([128, 128], BF16)
            make_identity(nc, ident)

            mtri = None
            if causal:
                # keep col >= row (upper triangle incl diagonal) of a
                # [s_local, t_local] 128x128 tile
                mtri = const.tile([128, 128], BF16)
                nc.vector.memset(mtri, 1.0)
                nc.gpsimd.affine_select(
                    out=mtri, in_=mtri, pattern=[[1, 128]], base=0,
                    channel_multiplier=-1, compare_op=mybir.AluOpType.is_ge,
                    fill=0.0)

            # ---------------- weights + x loads, spread over 3 queues ------
            wkv_sb = big.tile([128, KT, 256], BF16)
            nc.sync.dma_start(out=wkv_sb, in_=wkv.rearrange("(n p) c -> p n c", p=128))

            cexp_sb = const.tile([128, NTT, 64], F32)
            sexp_sb = const.tile([128, NTT, 64], F32)
            nc.scalar.dma_start(out=cexp_sb, in_=cexp.rearrange("(n p) c -> p n c", p=128))
            nc.scalar.dma_start(out=sexp_sb, in_=sexp.rearrange("(n p) c -> p n c", p=128))

            wq_sb = big.tile([128, KT, 512], BF16)
            nc.gpsimd.dma_start(out=wq_sb, in_=wq.rearrange("(n p) c -> p n c", p=128))

            xt_sb = big.tile([128, KT, T], BF16)
            xq = [nc.sync, nc.scalar, nc.gpsimd]

            wo_sb = big.tile([128, 4, D], BF16)

            kT_sb = big.tile([128, NTT, 128], BF16)
            qT_all = big.tile([128, NTT, 512], BF16)
            # [kv0 | 1 | kv1 | 1]: v features for both kv heads plus the
            # all-ones denominator columns, one copy per s-tile
            vp = big.tile([128, NTT, 2, 65], BF16)
            nc.vector.memset(vp[:, :, :, 64:65], 1.0)

            def rope(ps3, out_bf, ti, nh):
                """RoPE in [t, f] layout. ps3: PSUM [128, nh, 64] f32 view ->
                out_bf: SBUF [128, nh*64] bf16. One DVE op to drain PSUM
                fast; the arithmetic runs on gpsimd from SBUF."""
                o3 = out_bf.rearrange("p (h k) -> p h k", h=nh)
                tmp = ropew.tile([128, nh, 64], F32, tag="ropeT", name="rt")
                nc.vector.tensor_copy(tmp, ps3)
                a = ropew.tile([128, nh, 64], F32, tag="ropeA", name="ra")
                bt = ropew.tile([128, nh, 64], F32, tag="ropeB", name="rb")
                cb = _bcast(cexp_sb[:, ti, :], nh)
                nc.gpsimd.tensor_tensor(out=a, in0=tmp, in1=cb, op=mybir.AluOpType.mult)
                sb_lo = _bcast(sexp_sb[:, ti, 0:32], nh)
                sb_hi = _bcast(sexp_sb[:, ti, 32:64], nh)
                nc.gpsimd.tensor_tensor(out=bt[:, :, 0:32], in0=tmp[:, :, 32:64],
                                        in1=sb_lo, op=mybir.AluOpType.mult)
                nc.gpsimd.tensor_tensor(out=bt[:, :, 32:64], in0=tmp[:, :, 0:32],
                                        in1=sb_hi, op=mybir.AluOpType.mult)
                nc.gpsimd.tensor_tensor(out=o3, in0=a, in1=bt, op=mybir.AluOpType.add)

            # ------- projections for one quarter of the time axis ----------
            def load_quarter(c):
                lo, hi = c * 512, (c + 1) * 512
                for kt in range(KT):
                    xq[kt % 3].dma_start(out=xt_sb[:, kt, lo:hi],
                                         in_=xt[kt * 128:(kt + 1) * 128, lo:hi])
                if c == 0:
                    # wo is first needed by chunk 0's output projection; load
                    # it behind the quarter-0 x slices, spread over all queues
                    for g in range(4):
                        xq[g % 3].dma_start(
                            out=wo_sb[:, g, :], in_=wo[g * 128:(g + 1) * 128, :])

            def proj_tile_kv(st):
                ps_kv = pmm.tile([128, 256], F32, tag="mm", name="pskv")
                for kt in range(KT):
                    nc.tensor.matmul(ps_kv, xt_sb[:, kt, st * 128:(st + 1) * 128],
                                     wkv_sb[:, kt, :], start=(kt == 0),
                                     stop=(kt == KT - 1))
                ktf = qtfp.tile([128, 128], BF16, tag="qtf", name="ktf")
                rope(ps_kv[:, 0:128].rearrange("p (h k) -> p h k", h=2),
                     ktf, st, 2)
                nc.sync.dma_start_transpose(kT_sb[:, st, :], ktf)
                nc.vector.tensor_copy(
                    vp[:, st, :, 0:64],
                    ps_kv[:, 128:256].rearrange("p (h k) -> p h k", h=2))

            def proj_tile_q(ti):
                ps_q = pmm.tile([128, 512], F32, tag="mm", name="psq")
                for kt in range(KT):
                    nc.tensor.matmul(ps_q, xt_sb[:, kt, ti * 128:(ti + 1) * 128],
                                     wq_sb[:, kt, :], start=(kt == 0),
                                     stop=(kt == KT - 1))
                qtf = qtfp.tile([128, 512], BF16, tag="qtf", name="qtf")
                rope(ps_q.rearrange("p (h k) -> p h k", h=8), qtf, ti, 8)
                c, tt = ti // 4, ti % 4
                for g in range(4):
                    nc.sync.dma_start_transpose(
                        qT_all[:, 4 * c + g, tt * 128:(tt + 1) * 128],
                        qtf[:, g * 128:(g + 1) * 128])

            def proj_spliced(c, idx):
                """One projection tile of quarter c, spliced between the
                attention (g, half) units of chunk c-1 so the PE stream has
                filler while ACT grinds the exps."""
                if c >= NCH:
                    return
                if idx < 4:
                    proj_tile_kv(4 * c + idx)
                else:
                    proj_tile_q(4 * c + idx - 4)

            # ------- attention + output projection for one 512-wide chunk --
            def attention_chunk(j):
                if j + 1 < NCH:
                    load_quarter(j + 1)
                ohT_all = ohtp.tile([128, 4, 512], BF16, tag="oht", name="oht")
                for g in range(4):
                    qT = qT_all[:, 4 * j + g, :]
                    poht = None
                    for half in (0, 1):
                        hb = 64 * half
                        pav = pavp.tile([128, 4, 128], F32, tag="av", name="pav")
                        nc.vector.memset(pav[:, :, 0:65], 0.0)

                        def av(ptile, si, tb0):
                            for tb in range(tb0, 4):
                                nc.tensor.matmul(
                                    pav[:, tb, 0:65],
                                    ptile[:, (tb - tb0) * 128:(tb - tb0 + 1) * 128],
                                    vp[:, si, half, :], start=False, stop=False,
                                    skip_group_check=True)

                        ns_off = 4 * j if causal else NTT
                        for sp in range(0, ns_off, 2):
                            ps2 = psc.tile([128, 2, 512], F32, tag="sc", name="pss")
                            for u in (0, 1):
                                nc.tensor.matmul(
                                    ps2[:, u, :], kT_sb[hb:hb + 64, sp + u, :],
                                    qT[hb:hb + 64, :],
                                    start=True, stop=True, skip_group_check=True)
                            pt2 = ptp.tile([128, 2, 512], BF16, tag="pt", name="pt")
                            nc.scalar.activation(out=pt2, in_=ps2,
                                                 func=mybir.ActivationFunctionType.Exp,
                                                 scale=0.125)
                            for u in (0, 1):
                                av(pt2[:, u, :], sp + u, 0)

                        if causal:
                            s0 = 4 * j
                            psd1 = psc.tile([128, 2, 512], F32, tag="sc", name="psd1")
                            psd2 = psc.tile([128, 2, 512], F32, tag="sc", name="psd2")
                            nc.tensor.matmul(psd1[:, 0, :],
                                             kT_sb[hb:hb + 64, s0, :],
                                             qT[hb:hb + 64, :],
                                             start=True, stop=True,
                                             skip_group_check=True)
                            nc.tensor.matmul(psd1[:, 1, 0:384],
                                             kT_sb[hb:hb + 64, s0 + 1, :],
                                             qT[hb:hb + 64, 128:512],
                                             start=True, stop=True,
                                             skip_group_check=True)
                            nc.tensor.matmul(psd1[:, 1, 384:512],
                                             kT_sb[hb:hb + 64, s0 + 3, :],
                                             qT[hb:hb + 64, 384:512],
                                             start=True, stop=True,
                                             skip_group_check=True)
                            nc.tensor.matmul(psd2[:, 0, 0:256],
                                             kT_sb[hb:hb + 64, s0 + 2, :],
                                             qT[hb:hb + 64, 256:512],
                                             start=True, stop=True,
                                             skip_group_check=True)
                            pd1 = ptp.tile([128, 2, 512], BF16, tag="pt", name="pd1")
                            nc.scalar.activation(out=pd1, in_=psd1,
                                                 func=mybir.ActivationFunctionType.Exp,
                                                 scale=0.125)
                            pd2 = ptp.tile([128, 2, 512], BF16, tag="pt", name="pd2")
                            nc.scalar.activation(out=pd2[:, 0, 0:256],
                                                 in_=psd2[:, 0, 0:256],
                                                 func=mybir.ActivationFunctionType.Exp,
                                                 scale=0.125)
                            for msk in (pd1[:, 0, 0:128], pd1[:, 1, 0:128],
                                        pd1[:, 1, 384:512], pd2[:, 0, 0:128]):
                                nc.gpsimd.tensor_tensor(out=msk, in0=msk, in1=mtri,
                                                        op=mybir.AluOpType.mult)
                            av(pd1[:, 0, :], s0, 0)
                            av(pd1[:, 1, 0:384], s0 + 1, 1)
                            av(pd2[:, 0, 0:256], s0 + 2, 2)
                            av(pd1[:, 1, 384:512], s0 + 3, 3)

                        # softmax denominator -> reciprocal -> normalize
                        r4 = rp.tile([128, 4, 1], F32, tag="r4", name="r4")
                        nc.vector.reciprocal(out=r4, in_=pav[:, :, 64:65])
                        ohn = ohp.tile([128, 4, 64], BF16, tag="ohn", name="ohn")
                        nc.vector.tensor_tensor(out=ohn, in0=pav[:, :, 0:64],
                                                in1=_bcast_last(r4, 64),
                                                op=mybir.AluOpType.mult)
                        if poht is None:
                            poht = pmm.tile([128, 4, 128], BF16, tag="mm",
                                            name="poht")
                        for tb in range(4):
                            nc.tensor.transpose(poht[hb:hb + 64, tb, :],
                                                ohn[:, tb, :], ident)
                        proj_spliced(j + 1, 2 * g + half)
                    nc.vector.tensor_copy(
                        ohT_all[:, g, :].rearrange("p (a b) -> p a b", a=4), poht)

                # output projection for this t-chunk
                for jt in range(NTT):
                    ps_r = pmm.tile([128, 512], F32, tag="mm", name="psr")
                    for g in range(4):
                        nc.tensor.matmul(ps_r, wo_sb[:, g, jt * 128:(jt + 1) * 128],
                                         ohT_all[:, g, :], start=(g == 0),
                                         stop=(g == 3), skip_group_check=True)
                    rs = rsp.tile([128, 512], F32, tag="rs", name="rs")
                    nc.vector.tensor_copy(rs, ps_r)
                    eng = nc.sync if jt % 2 == 0 else nc.gpsimd
                    eng.dma_start(
                        out=res[jt * 128:(jt + 1) * 128, j * 512:(j + 1) * 512],
                        in_=rs)

            # Software-pipelined schedule: quarter 0 is projected up
            # front; afterwards, quarter j+1's projection tiles are spliced
            # between chunk j's attention units (see proj_spliced).
            load_quarter(0)
            for st in range(4):
                proj_tile_kv(st)
            for ti in range(4):
                proj_tile_q(ti)
            for j in range(NCH):
                attention_chunk(j)
    return nc


_NC_CACHE = {}


def _get_nc(causal: bool):
    if causal not in _NC_CACHE:
        _NC_CACHE[causal] = _build(causal)
    return _NC_CACHE[causal]


# ---------------------------------------------------------------------------
# Host wrapper
# ---------------------------------------------------------------------------

def kernel(x, cos, sin, mask, wq, wk, wv, wo):
    x = np.asarray(x, dtype=np.float32)
    cos = np.asarray(cos, dtype=np.float32)
    sin = np.asarray(sin, dtype=np.float32)
    mask = np.asarray(mask)
    wq = np.asarray(wq, dtype=np.float32)
    wk = np.asarray(wk, dtype=np.float32)
    wv = np.asarray(wv, dtype=np.float32)
    wo = np.asarray(wo, dtype=np.float32)

    m2 = mask[0, 0]
    tril = np.tril(np.ones((T, T), dtype=bool))
    if np.array_equal(m2, tril):
        causal = True
    elif m2.all():
        causal = False
    else:
        return _numpy_fallback(x, cos, sin, mask, wq, wk, wv, wo)

    _install_waitsplit()
    nc = _get_nc(causal)

    cexp = np.concatenate([cos, cos], axis=1).astype(np.float32)
    sexp = np.concatenate([-sin, sin], axis=1).astype(np.float32)

    in_maps = []
    for c in range(NCORES):
        b, jg = c // 4, c % 4
        heads = []
        for g in range(4):
            heads.append(8 * jg + g)
            heads.append(8 * jg + 4 + g)
        wq_rows = np.concatenate([wq[h * HD:(h + 1) * HD, :] for h in heads], axis=0)
        wo_cols = np.concatenate([wo[:, h * HD:(h + 1) * HD].T for h in heads], axis=0)
        kv = [2 * jg, 2 * jg + 1]
        wk_rows = np.concatenate([wk[k * HD:(k + 1) * HD, :] for k in kv], axis=0)
        wv_rows = np.concatenate([wv[k * HD:(k + 1) * HD, :] for k in kv], axis=0)
        wkv_cols = np.concatenate([wk_rows.T, wv_rows.T], axis=1)  # [D, 256]
        in_maps.append({
            "xt": np.ascontiguousarray(x[b].T).astype(NBF),
            "wq": np.ascontiguousarray(wq_rows.T).astype(NBF),
            "wkv": np.ascontiguousarray(wkv_cols).astype(NBF),
            "wo": np.ascontiguousarray(wo_cols).astype(NBF),
            "cexp": cexp,
            "sexp": sexp,
        })

    r = run_bass_kernel_spmd(nc, in_maps, core_ids=list(range(NCORES)))

    out = np.zeros((2, T, D), dtype=np.float32)
    for c in range(NCORES):
        out[c // 4] += r.results[c]["res"].T
    return out


def _numpy_fallback(x, cos, sin, mask, wq, wk, wv, wo):
    B = x.shape[0]
    NH, NKV = 32, 8
    q = (x @ wq.T).reshape(B, T, NH, HD).transpose(0, 2, 1, 3)
    k = (x @ wk.T).reshape(B, T, NKV, HD).transpose(0, 2, 1, 3)
    v = (x @ wv.T).reshape(B, T, NKV, HD).transpose(0, 2, 1, 3)

    def rope_np(t4):
        c = cos[None, None]
        s = sin[None, None]
        t1, t2 = t4[..., :32], t4[..., 32:]
        return np.concatenate([t1 * c - t2 * s, t2 * c + t1 * s], axis=-1)

    q, k = rope_np(q), rope_np(k)
    k = np.repeat(k, 4, axis=1)
    v = np.repeat(v, 4, axis=1)
    att = np.einsum("bhtd,bhsd->bhts", q, k) / np.sqrt(HD)
    att = np.where(mask, att, -np.inf)
    att = att - att.max(axis=-1, keepdims=True)
    p = np.exp(att)
    p /= p.sum(axis=-1, keepdims=True)
    o = np.einsum("bhts,bhsd->bhtd", p, v)
    o = o.transpose(0, 2, 1, 3).reshape(B, T, -1)
    return (o @ wo.T).astype(np.float32)


# revision 22
# speedup vs baseline: 1.3582x; 1.0029x over previous
"""GQA attention kernel for 8 Trainium2 cores (v2).

Problem: B=2, T=2048, D=2048, 32 q-heads, 8 kv-heads, head_dim=64, causal.

Sharding: core c = (b, jg) with b = c//4, jg = c%4. Each core handles batch b,
kv-heads {2jg, 2jg+1} and q-heads {8jg..8jg+7} (data parallel on B, tensor
parallel on heads; wq/wk/wv column-sharded, wo row-sharded). Each core returns
a partial output projection resT [D, T]; the host sums the 4 partials per
batch and transposes.

v2 design changes vs the 406us baseline:
 - input DMAs spread over the SP/ACT/POOL queues with wkv first so the first
   projection matmul starts at ~14us instead of ~44us.
 - k/q [t,f]->[f,t] transposes moved off the PE onto the DMA XBAR
   (dma_start_transpose), eliminating PE transpose+copy traffic.
 - AV matmul transposed: out[t(128), f(65)] accumulates with P-tiles as the
   stationary operand and [V|1] as the 65-row moving operand (65 rows vs 512
   rows per s-tile on the PE). The 65th column of the accumulator is the
   softmax denominator, so no separate reduction and no DRAM round-trip
   broadcast: reciprocal via ACT Ln/Exp on a [128,4,1] column, applied as a
   free-dim step-0 broadcast multiply.
 - PSUM accumulators for the 4 t-blocks share one bank; groups cannot use
   start=True (2KB zero-region granularity), so the bank is DVE-memset to 0
   and all AV matmuls accumulate with start=False.
 - causal diagonal computed at 128-column granularity (only the lower
   trapezoid), saving ~37% of diagonal scores/exp/AV work; only the true
   diagonal 128x128 subtiles get the multiplicative tril mask (on gpsimd).
 - output-projection results staged through SBUF (DMA cannot read PSUM) and
   streamed out per 128-row tile on the idle SP/POOL queues.
"""

import os
import sys

sys.path.insert(0, "/opt/trn_rl_repo")

import json

import numpy as np
import ml_dtypes

import concourse.bass as bass
import concourse.mybir as mybir
from concourse.tile import TileContext
from concourse.masks import make_identity
from concourse.bass_utils import run_bass_kernel_spmd

BF16 = mybir.dt.bfloat16
FP8 = mybir.dt.float8e4
F32 = mybir.dt.float32

T = 2048
D = 2048
HD = 64
NCORES = 8
KT = D // 128           # 16 contraction tiles
NTT = T // 128          # 16 time tiles
NCH = T // 512          # 4 chunks
NBF = ml_dtypes.bfloat16

# ---------------------------------------------------------------------------
# BIR post-pass: split multi-wait instructions into single-wait
# EventSemaphore carriers (the walrus build here allows one wait per inst).
# ---------------------------------------------------------------------------
_ws_ctr = [0]


def _split_waits_bytes(bir: bytes) -> bytes:
    d = json.loads(bir)
    for f in d.get("functions", []):
        for bb in f.get("blocks", []):
            out = []
            for inst in bb.get("instructions", []):
                si = inst.get("sync_info")
                waits = (si or {}).get("on_wait") or []
                if len(waits) > 1:
                    for w in waits[:-1]:
                        _ws_ctr[0] += 1
                        out.append({
                            "debug": inst.get("debug", 0),
                            "engine": inst["engine"],
                            "ins": [],
                            "name": f"WS-{_ws_ctr[0]}",
                            "opcode": "EventSemaphore",
                            "outs": [],
                            "sync_info": {"on_update": [], "on_wait": [w]},
                        })
                    si["on_wait"] = [waits[-1]]
                out.append(inst)
            bb["instructions"] = out
    return json.dumps(d).encode()


def _install_waitsplit():
    import concourse.bass2jax as b2j

    if getattr(b2j, "_waitsplit_installed", False):
        return
    orig = b2j._decompress_ant_bir
    b2j._decompress_ant_bir = lambda s: _split_waits_bytes(orig(s))
    b2j._waitsplit_installed = True


# ---------------------------------------------------------------------------
# Device program
# ---------------------------------------------------------------------------

def _bcast(ap2d, nh):
    """Insert a step-0 head dim into a [p, w] AP -> [p, nh, w]."""
    return bass.AP(tensor=ap2d.tensor, offset=ap2d.offset,
                   ap=[ap2d.ap[0], [0, nh], ap2d.ap[1]])


def _bcast_last(ap3d, w):
    """Append a step-0 last dim to a [p, n, 1] AP -> [p, n, w]."""
    return bass.AP(tensor=ap3d.tensor, offset=ap3d.offset,
                   ap=[ap3d.ap[0], ap3d.ap[1], [0, w]])


def _build(causal: bool):
    nc = bass.Bass()
    xt = nc.dram_tensor("xt", [D, T], BF16, kind="ExternalInput")
    wq = nc.dram_tensor("wq", [D, 512], BF16, kind="ExternalInput")
    wkv = nc.dram_tensor("wkv", [D, 256], BF16, kind="ExternalInput")
    wo = nc.dram_tensor("wo", [512, D], BF16, kind="ExternalInput")
    cexp = nc.dram_tensor("cexp", [T, 64], F32, kind="ExternalInput")
    sexp = nc.dram_tensor("sexp", [T, 64], F32, kind="ExternalInput")
    res = nc.dram_tensor("res", [D, T], F32, kind="ExternalOutput")

    with TileContext(nc) as tc:
        with (
            tc.tile_pool(name="const", bufs=1) as const,
            tc.tile_pool(name="big", bufs=1) as big,
            tc.tile_pool(name="ropew", bufs=3) as ropew,
            tc.tile_pool(name="qtfp", bufs=3) as qtfp,
            tc.tile_pool(name="ptp", bufs=4) as ptp,
            tc.tile_pool(name="ohp", bufs=3) as ohp,
            tc.tile_pool(name="rp", bufs=3) as rp,
            tc.tile_pool(name="ohtp", bufs=6) as ohtp,
            tc.tile_pool(name="rsp", bufs=3) as rsp,
            tc.tile_pool(name="pmm", bufs=2, space="PSUM") as pmm,
            tc.tile_pool(name="psc", bufs=2, space="PSUM") as psc,
            tc.tile_pool(name="pav", bufs=2, space="PSUM") as pavp,
        ):
            # ---------------- constants ----------------
            ident = const.tile([128, 128], BF16)
            make_identity(nc, ident)

            mtri = None
            if causal:
                # keep col >= row (upper triangle incl diagonal) of a
                # [s_local, t_local] 128x128 tile
                mtri = const.tile([128, 128], BF16)
                nc.vector.memset(mtri, 1.0)
                nc.gpsimd.affine_select(
                    out=mtri, in_=mtri, pattern=[[1, 128]], base=0,
                    channel_multiplier=-1, compare_op=mybir.AluOpType.is_ge,
                    fill=0.0)

            # ---------------- weights + x loads, spread over 3 queues ------
            wkv_sb = big.tile([128, KT, 256], BF16)
            nc.sync.dma_start(out=wkv_sb, in_=wkv.rearrange("(n p) c -> p n c", p=128))

            cexp_sb = const.tile([128, NTT, 64], F32)
            sexp_sb = const.tile([128, NTT, 64], F32)
            nc.scalar.dma_start(out=cexp_sb, in_=cexp.rearrange("(n p) c -> p n c", p=128))
            nc.scalar.dma_start(out=sexp_sb, in_=sexp.rearrange("(n p) c -> p n c", p=128))

            wq_sb = big.tile([128, KT, 512], BF16)
            nc.gpsimd.dma_start(out=wq_sb, in_=wq.rearrange("(n p) c -> p n c", p=128))

            xt_sb = big.tile([128, KT, T], BF16)
            xq = [nc.sync, nc.scalar, nc.gpsimd]

            wo_sb = big.tile([128, 4, D], BF16)

            kT_sb = big.tile([128, NTT, 128], BF16)
            qT_all = big.tile([128, NTT, 512], BF16)
            # [kv0 | 1 | kv1 | 1]: v features for both kv heads plus the
            # all-ones denominator columns, one copy per s-tile
            vp = big.tile([128, NTT, 2, 65], BF16)
            nc.vector.memset(vp[:, :, :, 64:65], 1.0)

            def rope(ps3, out_bf, ti, nh):
                """RoPE in [t, f] layout. ps3: PSUM [128, nh, 64] f32 view ->
                out_bf: SBUF [128, nh*64] bf16. One DVE op to drain PSUM
                fast; the arithmetic runs on gpsimd from SBUF."""
                o3 = out_bf.rearrange("p (h k) -> p h k", h=nh)
                tmp = ropew.tile([128, nh, 64], F32, tag="ropeT", name="rt")
                nc.vector.tensor_copy(tmp, ps3)
                a = ropew.tile([128, nh, 64], F32, tag="ropeA", name="ra")
                bt = ropew.tile([128, nh, 64], F32, tag="ropeB", name="rb")
                cb = _bcast(cexp_sb[:, ti, :], nh)
                nc.gpsimd.tensor_tensor(out=a, in0=tmp, in1=cb, op=mybir.AluOpType.mult)
                sb_lo = _bcast(sexp_sb[:, ti, 0:32], nh)
                sb_hi = _bcast(sexp_sb[:, ti, 32:64], nh)
                nc.gpsimd.tensor_tensor(out=bt[:, :, 0:32], in0=tmp[:, :, 32:64],
                                        in1=sb_lo, op=mybir.AluOpType.mult)
                nc.gpsimd.tensor_tensor(out=bt[:, :, 32:64], in0=tmp[:, :, 0:32],
                                        in1=sb_hi, op=mybir.AluOpType.mult)
                nc.gpsimd.tensor_tensor(out=o3, in0=a, in1=bt, op=mybir.AluOpType.add)

            # ------- projections for one quarter of the time axis ----------
            def load_quarter(c):
                lo, hi = c * 512, (c + 1) * 512
                for kt in range(KT):
                    xq[kt % 3].dma_start(out=xt_sb[:, kt, lo:hi],
                                         in_=xt[kt * 128:(kt + 1) * 128, lo:hi])
                if c == 0:
                    # wo is first needed by chunk 0's output projection; load
                    # it behind the quarter-0 x slices, spread over all queues
                    for g in range(4):
                        xq[g % 3].dma_start(
                            out=wo_sb[:, g, :], in_=wo[g * 128:(g + 1) * 128, :])

            def proj_tile_kv(st):
                ps_kv = pmm.tile([128, 256], F32, tag="mm", name="pskv")
                for kt in range(KT):
                    nc.tensor.matmul(ps_kv, xt_sb[:, kt, st * 128:(st + 1) * 128],
                                     wkv_sb[:, kt, :], start=(kt == 0),
                                     stop=(kt == KT - 1))
                ktf = qtfp.tile([128, 128], BF16, tag="qtf", name="ktf")
                rope(ps_kv[:, 0:128].rearrange("p (h k) -> p h k", h=2),
                     ktf, st, 2)
                nc.sync.dma_start_transpose(kT_sb[:, st, :], ktf)
                nc.vector.tensor_copy(
                    vp[:, st, :, 0:64],
                    ps_kv[:, 128:256].rearrange("p (h k) -> p h k", h=2))

            def proj_tile_q(ti):
                ps_q = pmm.tile([128, 512], F32, tag="mm", name="psq")
                for kt in range(KT):
                    nc.tensor.matmul(ps_q, xt_sb[:, kt, ti * 128:(ti + 1) * 128],
                                     wq_sb[:, kt, :], start=(kt == 0),
                                     stop=(kt == KT - 1))
                qtf = qtfp.tile([128, 512], BF16, tag="qtf", name="qtf")
                rope(ps_q.rearrange("p (h k) -> p h k", h=8), qtf, ti, 8)
                c, tt = ti // 4, ti % 4
                for g in range(4):
                    nc.sync.dma_start_transpose(
                        qT_all[:, 4 * c + g, tt * 128:(tt + 1) * 128],
                        qtf[:, g * 128:(g + 1) * 128])


            # ------- attention + output projection for one 512-wide chunk --
            def attention_chunk(j, splice_q):
                ohT_all = ohtp.tile([128, 4, 512], BF16, tag="oht", name="oht")
                for g in range(4):
                    qT = qT_all[:, 4 * j + g, :]
                    poht = None
                    for half in (0, 1):
                        hb = 64 * half
                        pav = pavp.tile([128, 4, 128], F32, tag="av", name="pav")
                        nc.vector.memset(pav[:, :, 0:65], 0.0)

                        def av(ptile, si, tb0):
                            for tb in range(tb0, 4):
                                nc.tensor.matmul(
                                    pav[:, tb, 0:65],
                                    ptile[:, (tb - tb0) * 128:(tb - tb0 + 1) * 128],
                                    vp[:, si, half, :], start=False, stop=False,
                                    skip_group_check=True)

                        ns_off = 4 * j if causal else NTT
                        for sp in range(0, ns_off, 2):
                            ps2 = psc.tile([128, 2, 512], F32, tag="sc", name="pss")
                            for u in (0, 1):
                                nc.tensor.matmul(
                                    ps2[:, u, :], kT_sb[hb:hb + 64, sp + u, :],
                                    qT[hb:hb + 64, :],
                                    start=True, stop=True, skip_group_check=True)
                            pt2 = ptp.tile([128, 2, 512], BF16, tag="pt", name="pt")
                            nc.scalar.activation(out=pt2, in_=ps2,
                                                 func=mybir.ActivationFunctionType.Exp,
                                                 scale=0.125)
                            for u in (0, 1):
                                av(pt2[:, u, :], sp + u, 0)

                        if causal:
                            s0 = 4 * j
                            psd1 = psc.tile([128, 2, 512], F32, tag="sc", name="psd1")
                            psd2 = psc.tile([128, 2, 512], F32, tag="sc", name="psd2")
                            nc.tensor.matmul(psd1[:, 0, :],
                                             kT_sb[hb:hb + 64, s0, :],
                                             qT[hb:hb + 64, :],
                                             start=True, stop=True,
                                             skip_group_check=True)
                            nc.tensor.matmul(psd1[:, 1, 0:384],
                                             kT_sb[hb:hb + 64, s0 + 1, :],
                                             qT[hb:hb + 64, 128:512],
                                             start=True, stop=True,
                                             skip_group_check=True)
                            nc.tensor.matmul(psd1[:, 1, 384:512],
                                             kT_sb[hb:hb + 64, s0 + 3, :],
                                             qT[hb:hb + 64, 384:512],
                                             start=True, stop=True,
                                             skip_group_check=True)
                            nc.tensor.matmul(psd2[:, 0, 0:256],
                                             kT_sb[hb:hb + 64, s0 + 2, :],
                                             qT[hb:hb + 64, 256:512],
                                             start=True, stop=True,
                                             skip_group_check=True)
                            pd1 = ptp.tile([128, 2, 512], BF16, tag="pt", name="pd1")
                            nc.scalar.activation(out=pd1, in_=psd1,
                                                 func=mybir.ActivationFunctionType.Exp,
                                                 scale=0.125)
                            pd2 = ptp.tile([128, 2, 512], BF16, tag="pt", name="pd2")
                            nc.scalar.activation(out=pd2[:, 0, 0:256],
                                                 in_=psd2[:, 0, 0:256],
                                                 func=mybir.ActivationFunctionType.Exp,
                                                 scale=0.125)
                            for msk in (pd1[:, 0, 0:128], pd1[:, 1, 0:128],
                                        pd1[:, 1, 384:512], pd2[:, 0, 0:128]):
                                nc.gpsimd.tensor_tensor(out=msk, in0=msk, in1=mtri,
                                                        op=mybir.AluOpType.mult)
                            av(pd1[:, 0, :], s0, 0)
                            av(pd1[:, 1, 0:384], s0 + 1, 1)
                            av(pd2[:, 0, 0:256], s0 + 2, 2)
                            av(pd1[:, 1, 384:512], s0 + 3, 3)

                        # softmax denominator -> reciprocal -> normalize
                        r4 = rp.tile([128, 4, 1], F32, tag="r4", name="r4")
                        nc.vector.reciprocal(out=r4, in_=pav[:, :, 64:65])
                        ohn = ohp.tile([128, 4, 64], BF16, tag="ohn", name="ohn")
                        nc.vector.tensor_tensor(out=ohn, in0=pav[:, :, 0:64],
                                                in1=_bcast_last(r4, 64),
                                                op=mybir.AluOpType.mult)
                        if poht is None:
                            poht = pmm.tile([128, 4, 128], BF16, tag="mm",
                                            name="poht")
                        for tb in range(4):
                            nc.tensor.transpose(poht[hb:hb + 64, tb, :],
                                                ohn[:, tb, :], ident)
                        if 2 * g + half < len(splice_q):
                            proj_tile_q(splice_q[2 * g + half])
                    nc.vector.tensor_copy(
                        ohT_all[:, g, :].rearrange("p (a b) -> p a b", a=4), poht)

                # output projection for this t-chunk
                for jt in range(NTT):
                    ps_r = pmm.tile([128, 512], F32, tag="mm", name="psr")
                    for g in range(4):
                        nc.tensor.matmul(ps_r, wo_sb[:, g, jt * 128:(jt + 1) * 128],
                                         ohT_all[:, g, :], start=(g == 0),
                                         stop=(g == 3), skip_group_check=True)
                    rs = rsp.tile([128, 512], F32, tag="rs", name="rs")
                    nc.vector.tensor_copy(rs, ps_r)
                    eng = nc.sync if jt % 2 == 0 else nc.gpsimd
                    eng.dma_start(
                        out=res[jt * 128:(jt + 1) * 128, j * 512:(j + 1) * 512],
                        in_=rs)

            # Reversed-chunk schedule: all kv tiles plus quarter-3 q are
            # projected first so the largest chunk (3) -- which carries 40%
            # of the exp work -- starts as early as possible and ACT is
            # front-loaded. The remaining q projections are spliced between
            # attention (g, half) units as PE filler during exp-bound
            # stretches, ordered so each later chunk finds its q ready.
            for c in range(NCH):
                load_quarter(c)
            for st in range(NTT):
                proj_tile_kv(st)
            for ti in range(12, 16):
                proj_tile_q(ti)
            attention_chunk(3, [8, 9, 10, 11, 4, 5, 6, 7])
            attention_chunk(2, [0, 1, 2, 3])
            attention_chunk(1, [])
            attention_chunk(0, [])
    return nc


_NC_CACHE = {}


def _get_nc(causal: bool):
    if causal not in _NC_CACHE:
        _NC_CACHE[causal] = _build(causal)
    return _NC_CACHE[causal]


# ---------------------------------------------------------------------------
# Host wrapper
# ---------------------------------------------------------------------------

def kernel(x, cos, sin, mask, wq, wk, wv, wo):
    x = np.asarray(x, dtype=np.float32)
    cos = np.asarray(cos, dtype=np.float32)
    sin = np.asarray(sin, dtype=np.float32)
    mask = np.asarray(mask)
    wq = np.asarray(wq, dtype=np.float32)
    wk = np.asarray(wk, dtype=np.float32)
    wv = np.asarray(wv, dtype=np.float32)
    wo = np.asarray(wo, dtype=np.float32)

    m2 = mask[0, 0]
    tril = np.tril(np.ones((T, T), dtype=bool))
    if np.array_equal(m2, tril):
        causal = True
    elif m2.all():
        causal = False
    else:
        return _numpy_fallback(x, cos, sin, mask, wq, wk, wv, wo)

    _install_waitsplit()
    nc = _get_nc(causal)

    cexp = np.concatenate([cos, cos], axis=1).astype(np.float32)
    sexp = np.concatenate([-sin, sin], axis=1).astype(np.float32)

    in_maps = []
    for c in range(NCORES):
        b, jg = c // 4, c % 4
        heads = []
        for g in range(4):
            heads.append(8 * jg + g)
            heads.append(8 * jg + 4 + g)
        wq_rows = np.concatenate([wq[h * HD:(h + 1) * HD, :] for h in heads], axis=0)
        wo_cols = np.concatenate([wo[:, h * HD:(h + 1) * HD].T for h in heads], axis=0)
        kv = [2 * jg, 2 * jg + 1]
        wk_rows = np.concatenate([wk[k * HD:(k + 1) * HD, :] for k in kv], axis=0)
        wv_rows = np.concatenate([wv[k * HD:(k + 1) * HD, :] for k in kv], axis=0)
        wkv_cols = np.concatenate([wk_rows.T, wv_rows.T], axis=1)  # [D, 256]
        in_maps.append({
            "xt": np.ascontiguousarray(x[b].T).astype(NBF),
            "wq": np.ascontiguousarray(wq_rows.T).astype(NBF),
            "wkv": np.ascontiguousarray(wkv_cols).astype(NBF),
            "wo": np.ascontiguousarray(wo_cols).astype(NBF),
            "cexp": cexp,
            "sexp": sexp,
        })

    r = run_bass_kernel_spmd(nc, in_maps, core_ids=list(range(NCORES)))

    out = np.zeros((2, T, D), dtype=np.float32)
    for c in range(NCORES):
        out[c // 4] += r.results[c]["res"].T
    return out


def _numpy_fallback(x, cos, sin, mask, wq, wk, wv, wo):
    B = x.shape[0]
    NH, NKV = 32, 8
    q = (x @ wq.T).reshape(B, T, NH, HD).transpose(0, 2, 1, 3)
    k = (x @ wk.T).reshape(B, T, NKV, HD).transpose(0, 2, 1, 3)
    v = (x @ wv.T).reshape(B, T, NKV, HD).transpose(0, 2, 1, 3)

    def rope_np(t4):
        c = cos[None, None]
        s = sin[None, None]
        t1, t2 = t4[..., :32], t4[..., 32:]
        return np.concatenate([t1 * c - t2 * s, t2 * c + t1 * s], axis=-1)

    q, k = rope_np(q), rope_np(k)
    k = np.repeat(k, 4, axis=1)
    v = np.repeat(v, 4, axis=1)
    att = np.einsum("bhtd,bhsd->bhts", q, k) / np.sqrt(HD)
    att = np.where(mask, att, -np.inf)
    att = att - att.max(axis=-1, keepdims=True)
    p = np.exp(att)
    p /= p.sum(axis=-1, keepdims=True)
    o = np.einsum("bhts,bhsd->bhtd", p, v)
    o = o.transpose(0, 2, 1, 3).reshape(B, T, -1)
    return (o @ wo.T).astype(np.float32)
